# revision 1
# baseline (speedup 1.0000x reference)
"""GAT layer (LayerNorm -> GATConv(heads=1) -> residual ReLU) on 8 trn2 NeuronCores.

Sharding: destination-node (graph/data) parallel. Each core owns a contiguous
range of N/8 nodes: it computes the node transform for its shard, the shards
are AllGathered so every core holds the full transformed-node table, and each
core then processes the edges whose destination falls in its shard.

Per destination block of 128 nodes, source-node records are fetched with
dma_gather (768 B rows: [xp+bias | 1 | a_src | pad]), per-edge a_dst with a
second dma_gather from a core-local 256 B-row table, attention weights
ee = exp(leakyrelu(a_src + a_dst)) are computed on DVE/ACT, and the
scatter-add is a one-hot matmul: lhsT[e, r] = (iota_r == dstlocal_e) * ee_e
accumulated into PSUM; the table's ones-column yields the softmax denominator
in the same matmuls. Attention/norm parameters are folded on the host into a
single [D,131] matrix + affine row and replicated to every core.
"""

import numpy as np

import concourse.bacc as bacc
import concourse.mybir as mybir
import concourse.tile as tile
from concourse.bass_utils import run_bass_kernel_spmd

F32 = mybir.dt.float32
I16 = mybir.dt.int16
AX = mybir.AxisListType
OP = mybir.AluOpType
AF = mybir.ActivationFunctionType

N = 50000
D = 128
E = 600000
NCORES = 8
SHARD = N // NCORES            # 6250
NBLK = (SHARD + 127) // 128    # 49 dst blocks per core
PAD_SHARD = NBLK * 128         # 6272
LAST_ROWS = SHARD - (NBLK - 1) * 128  # 106
FROW = 192                     # table row f32s (768 B, dma_gather granularity)
AROW = 64                      # a_dst table row f32s (256 B)
GCOL = 130                     # matmul rhs columns: [feat(128) | 1 | a_src]
COL_ONE = 128
COL_ASRC = 129
HALF = 32768                   # int16 index split point for the global table
NEG_SLOPE = 0.2
LN_EPS = 1e-5
GBLK = 2                       # dst blocks per gather group
DEBUG_MAX_GROUPS = None        # limit phase-B groups (bisection aid)
DEBUG_STAGE = 4                # 1=gathers 2=+ee 3=+matmul 4=full (bisection aid)
DEBUG_NO_AG = False            # replace AllGather with local copy (bisection aid)
DEBUG_NO_PHASE_A = False       # stub out phase-A compute (bisection aid)
DEBUG_GATHERS = "both"         # "feat" | "adst" | "both" (bisection aid)


def _build_program(tlo, thi):
    """One SPMD program; per-core behaviour differs only through its inputs.

    tlo/thi: per-block tile counts (of 128 edge slots) for the low/high
    halves of the source table, uniform across cores.
    """
    nc = bacc.Bacc("TRN2", num_devices=NCORES, debug=False)

    CB = sum(tlo) + sum(thi)   # total column-blocks (tiles) per core

    x_shard = nc.dram_tensor("x_shard", [PAD_SHARD, D], F32, kind="ExternalInput")
    wext = nc.dram_tensor("wext", [D, 131], F32, kind="ExternalInput")
    c2b = nc.dram_tensor("c2b", [128, 131], F32, kind="ExternalInput")
    ident = nc.dram_tensor("ident", [128, 128], F32, kind="ExternalInput")
    iota = nc.dram_tensor("iota", [128, 128], F32, kind="ExternalInput")
    feat_idx = nc.dram_tensor("feat_idx", [128, CB * 8], I16, kind="ExternalInput")
    adst_idx = nc.dram_tensor("adst_idx", [128, CB * 8], I16, kind="ExternalInput")
    dloc = nc.dram_tensor("dloc", [128, CB], F32, kind="ExternalInput")
    out_shard = nc.dram_tensor("out_shard", [SHARD, D], F32, kind="ExternalOutput")

    # group structure (static, identical on every core)
    groups = []
    cb0 = 0
    for g0 in range(0, NBLK, GBLK):
        blocks = list(range(g0, min(NBLK, g0 + GBLK)))
        nlo = sum(tlo[b] for b in blocks)
        nhi = sum(thi[b] for b in blocks)
        groups.append((blocks, cb0, nlo, nhi))
        cb0 += nlo + nhi
    assert cb0 == CB
    CBG_MAX = max(nlo + nhi for _, _, nlo, nhi in groups)

    with tile.TileContext(nc) as tc:
        with (
            tc.tile_pool(name="dram", bufs=1, space="DRAM") as dram,
            tc.tile_pool(name="consts", bufs=1) as cpool,
            tc.tile_pool(name="xres", bufs=1) as xpool,
        ):
            xp_shard = dram.tile([SHARD, FROW], F32)
            xp_full = dram.tile([N, FROW], F32, addr_space="Shared")
            adst_loc = dram.tile([PAD_SHARD, AROW], F32)

            ident_sb = cpool.tile([128, 128], F32)
            nc.sync.dma_start(ident_sb[:], ident[:, :])
            iota_sb = cpool.tile([128, 128], F32)
            nc.sync.dma_start(iota_sb[:], iota[:, :])
            wext_sb = cpool.tile([D, 131], F32)
            nc.sync.dma_start(wext_sb[:], wext[:, :])
            c2b_sb = cpool.tile([128, 131], F32)
            nc.sync.dma_start(c2b_sb[:], c2b[:, :])
            eps_sb = cpool.tile([128, 1], F32)
            nc.vector.memset(eps_sb[:], LN_EPS)
            fidx_sb = cpool.tile([128, CB * 8], I16)
            nc.sync.dma_start(fidx_sb[:], feat_idx[:, :])
            aidx_sb = cpool.tile([128, CB * 8], I16)
            nc.sync.dma_start(aidx_sb[:], adst_idx[:, :])
            dl_sb = cpool.tile([128, CB], F32)
            nc.sync.dma_start(dl_sb[:], dloc[:, :])

            x_tiles = []
            for i in range(NBLK):
                xt = xpool.tile([128, D], F32, tag=f"xres{i}")
                nc.sync.dma_start(xt[:], x_shard[i * 128 : (i + 1) * 128, :])
                x_tiles.append(xt)

            # ---------------- Phase A: node transform on own shard ---------
            if DEBUG_NO_PHASE_A:
                nc.sync.dma_start(xp_shard[:, 0:D], x_shard[0:SHARD, :])
                nc.sync.dma_start(
                    adst_loc[0:SHARD, 0:1], x_shard[0:SHARD, 0:1]
                )
            with (
                tc.tile_pool(name="a_small", bufs=8) as spool,
                tc.tile_pool(name="a_sq", bufs=2) as sqpool,
                tc.tile_pool(name="a_xnp", bufs=3) as xnppool,
                tc.tile_pool(name="a_xnpT", bufs=3) as xnptpool,
                tc.tile_pool(name="a_xpe", bufs=3) as xpepool,
                tc.tile_pool(name="a_ps_t", bufs=2, space="PSUM") as psa,
                tc.tile_pool(name="a_ps_m", bufs=2, space="PSUM") as psb,
            ):
                for i in range(NBLK if not DEBUG_NO_PHASE_A else 0):
                    xt = x_tiles[i]
                    rows = 128 if i < NBLK - 1 else LAST_ROWS
                    sumx = spool.tile([128, 1], F32, tag="sumx")
                    nc.vector.tensor_reduce(sumx[:], xt[:], AX.X, OP.add)
                    sqj = sqpool.tile([128, D], F32)
                    ssq = spool.tile([128, 1], F32, tag="ssq")
                    nc.scalar.activation(sqj[:], xt[:], AF.Square, accum_out=ssq[:])
                    mu = spool.tile([128, 1], F32, tag="mu")
                    nc.vector.tensor_scalar(mu[:], sumx[:], 1.0 / D, None, OP.mult)
                    m2 = spool.tile([128, 1], F32, tag="m2")
                    nc.vector.tensor_tensor(m2[:], mu[:], mu[:], OP.mult)
                    var = spool.tile([128, 1], F32, tag="var")
                    nc.vector.tensor_scalar(
                        var[:], ssq[:], 1.0 / D, m2[:, 0:1], OP.mult, OP.subtract
                    )
                    std = spool.tile([128, 1], F32, tag="std")
                    nc.scalar.activation(std[:], var[:], AF.Sqrt, bias=eps_sb[:, 0:1])
                    rstd = spool.tile([128, 1], F32, tag="rstd")
                    nc.vector.reciprocal(rstd[:], std[:])
                    xnp = xnppool.tile([128, D], F32)
                    nc.vector.tensor_scalar(
                        xnp[:], xt[:], mu[:, 0:1], rstd[:, 0:1], OP.subtract, OP.mult
                    )
                    pt = psa.tile([128, 128], F32, space="PSUM")
                    nc.tensor.transpose(pt[:], xnp[:], ident_sb[:])
                    xnpT = xnptpool.tile([128, 128], F32)
                    nc.scalar.copy(xnpT[:], pt[:])
                    pm = psb.tile([128, 131], F32, space="PSUM")
                    nc.tensor.matmul(
                        pm[:], lhsT=xnpT[:], rhs=wext_sb[:], start=True, stop=True
                    )
                    xpe = xpepool.tile([128, 131], F32)
                    nc.vector.tensor_tensor(xpe[:], pm[:], c2b_sb[:], OP.add)
                    nc.sync.dma_start(
                        xp_shard[i * 128 : i * 128 + rows, 0:130], xpe[:rows, 0:130]
                    )
                    nc.sync.dma_start(
                        adst_loc[i * 128 : i * 128 + rows, 0:1], xpe[:rows, 130:131]
                    )

            if DEBUG_NO_AG:
                nc.sync.dma_start(xp_full[0:SHARD, :], xp_shard[:, :])
            else:
                nc.gpsimd.collective_compute(
                    "AllGather",
                    OP.bypass,
                    replica_groups=[list(range(NCORES))],
                    ins=[xp_shard[:, :]],
                    outs=[xp_full[:, :]],
                )

            # ---------------- Phase B: edge aggregation --------------------
            with (
                tc.tile_pool(name="b_g", bufs=2) as gpool,
                tc.tile_pool(name="b_a", bufs=2) as apool,
                tc.tile_pool(name="b_sw", bufs=4) as swpool,
                tc.tile_pool(name="b_e", bufs=3) as epool,
                tc.tile_pool(name="b_ep", bufs=3) as eppool,
                tc.tile_pool(name="b_ps", bufs=4, space="PSUM") as psc,
            ):
                use_groups = groups if DEBUG_MAX_GROUPS is None else groups[:DEBUG_MAX_GROUPS]
                for blocks, cb0, nlo, nhi in use_groups:
                    cbg = nlo + nhi
                    gf = gpool.tile([128, CBG_MAX, FROW], F32, tag="gf")
                    if DEBUG_GATHERS == "adst":
                        nc.vector.memset(gf.rearrange("p a b -> p (a b)")[:], 0.0)
                    if nlo and DEBUG_GATHERS in ("feat", "both"):
                        nc.gpsimd.dma_gather(
                            out_ap=gf[:, 0:nlo, :],
                            in_ap=xp_full[0:HALF, :],
                            idxs_ap=fidx_sb[:, cb0 * 8 : (cb0 + nlo) * 8],
                            num_idxs=nlo * 128,
                            num_idxs_reg=nlo * 128,
                            elem_size=FROW,
                            single_packet=False,
                        )
                    if nhi and DEBUG_GATHERS in ("feat", "both"):
                        nc.gpsimd.dma_gather(
                            out_ap=gf[:, nlo:cbg, :],
                            in_ap=xp_full[HALF:N, :],
                            idxs_ap=fidx_sb[:, (cb0 + nlo) * 8 : (cb0 + cbg) * 8],
                            num_idxs=nhi * 128,
                            num_idxs_reg=nhi * 128,
                            elem_size=FROW,
                            single_packet=False,
                        )
                    ga = apool.tile([128, CBG_MAX, AROW], F32, tag="ga")
                    if DEBUG_GATHERS == "feat":
                        nc.vector.memset(ga.rearrange("p a b -> p (a b)")[:], 1.0)
                    if DEBUG_GATHERS in ("adst", "both"):
                      nc.gpsimd.dma_gather(
                        out_ap=ga[:, 0:cbg, :],
                        in_ap=adst_loc[:, :],
                        idxs_ap=aidx_sb[:, cb0 * 8 : (cb0 + cbg) * 8],
                        num_idxs=cbg * 128,
                        num_idxs_reg=cbg * 128,
                        elem_size=AROW,
                        single_packet=False,
                    )
                    if DEBUG_STAGE < 2:
                        for b in blocks:
                            rows = 128 if b < NBLK - 1 else LAST_ROWS
                            nc.sync.dma_start(
                                out_shard[b * 128 : b * 128 + rows, :],
                                gf[:rows, (b - blocks[0]), 0:D],
                            )
                        continue
                    # ee = exp(leakyrelu(a_src + a_dst)) for the whole group
                    e1 = epool.tile([128, CBG_MAX], F32, tag="e1")
                    nc.vector.tensor_tensor(
                        e1[:, 0:cbg], gf[:, 0:cbg, COL_ASRC], ga[:, 0:cbg, 0], OP.add
                    )
                    e2 = epool.tile([128, CBG_MAX], F32, tag="e2")
                    nc.vector.tensor_scalar(
                        e2[:, 0:cbg], e1[:, 0:cbg], NEG_SLOPE, None, OP.mult
                    )
                    e3 = epool.tile([128, CBG_MAX], F32, tag="e3")
                    nc.vector.tensor_tensor(
                        e3[:, 0:cbg], e2[:, 0:cbg], e1[:, 0:cbg], OP.max
                    )
                    ee = epool.tile([128, CBG_MAX], F32, tag="ee")
                    nc.scalar.activation(ee[:, 0:cbg], e3[:, 0:cbg], AF.Exp)
                    if DEBUG_STAGE < 3:
                        for b in blocks:
                            rows = 128 if b < NBLK - 1 else LAST_ROWS
                            tmp = eppool.tile([128, D], F32, tag="outt")
                            nc.vector.tensor_scalar(
                                tmp[:], iota_sb[:],
                                ee[:, (b - blocks[0]) : (b - blocks[0]) + 1],
                                None, OP.mult,
                            )
                            nc.sync.dma_start(
                                out_shard[b * 128 : b * 128 + rows, :], tmp[:rows, :]
                            )
                        continue

                    # per-block one-hot scatter matmuls
                    lo_off = 0
                    hi_off = nlo
                    for b in blocks:
                        rows = 128 if b < NBLK - 1 else LAST_ROWS
                        cbs = list(range(lo_off, lo_off + tlo[b])) + list(
                            range(hi_off, hi_off + thi[b])
                        )
                        lo_off += tlo[b]
                        hi_off += thi[b]
                        ps = psc.tile([128, GCOL], F32, space="PSUM")
                        for j, cb in enumerate(cbs):
                            sw = swpool.tile([128, 128], F32)
                            nc.vector.tensor_scalar(
                                sw[:],
                                iota_sb[:],
                                dl_sb[:, cb0 + cb : cb0 + cb + 1],
                                ee[:, cb : cb + 1],
                                OP.is_equal,
                                OP.mult,
                            )
                            nc.tensor.matmul(
                                ps[:, :],
                                lhsT=sw[:],
                                rhs=gf[:, cb, 0:GCOL],
                                start=(j == 0),
                                stop=(j == len(cbs) - 1),
                            )
                        if DEBUG_STAGE < 4:
                            tmp = eppool.tile([128, D], F32, tag="outt")
                            nc.vector.tensor_copy(tmp[:], ps[:, 0:D])
                            nc.sync.dma_start(
                                out_shard[b * 128 : b * 128 + rows, :], tmp[:rows, :]
                            )
                            continue
                        recip = epool.tile([128, 1], F32, tag="recip")
                        nc.vector.reciprocal(recip[:], ps[:, COL_ONE : COL_ONE + 1])
                        scaled = eppool.tile([128, D], F32, tag="scaled")
                        nc.scalar.activation(
                            scaled[:], ps[:, 0:D], AF.Copy, scale=recip[:, 0:1]
                        )
                        resid = eppool.tile([128, D], F32, tag="resid")
                        nc.vector.tensor_tensor(
                            resid[:], scaled[:], x_tiles[b][:], OP.add
                        )
                        outt = eppool.tile([128, D], F32, tag="outt")
                        nc.scalar.activation(outt[:], resid[:], AF.Relu)
                        nc.sync.dma_start(
                            out_shard[b * 128 : b * 128 + rows, :], outt[:rows, :]
                        )

    nc.compile()
    return nc


def _wrap_idx(idx):
    """int16 index list -> dma_gather SBUF layout [128, len/16]:
    index i lives at partitions {16g + i%16: g in 0..7}, column i//16."""
    L = len(idx)
    assert L % 16 == 0
    w = idx.reshape(L // 16, 16).T.astype(np.int16)      # [16, L/16]
    return np.tile(w, (8, 1))                            # [128, L/16]


def _host_prep(x, edge_index, ln_gamma, ln_beta, W, att_src, att_dst, bias):
    """Fold parameters and bucket edges by destination block. Numpy only."""
    Wt = W.T.astype(np.float64)
    G = ln_gamma.astype(np.float64)[:, None] * Wt          # [D, D]
    crow = ln_beta.astype(np.float64) @ Wt                 # [D]
    v_src = G @ att_src.astype(np.float64)
    v_dst = G @ att_dst.astype(np.float64)
    c_src = float(crow @ att_src.astype(np.float64))
    c_dst = float(crow @ att_dst.astype(np.float64))

    wext = np.zeros((D, 131), np.float32)
    wext[:, 0:D] = G.astype(np.float32)
    wext[:, COL_ASRC] = v_src.astype(np.float32)
    wext[:, 130] = v_dst.astype(np.float32)
    c2 = np.zeros((131,), np.float32)
    c2[0:D] = (crow + bias.astype(np.float64)).astype(np.float32)
    c2[COL_ONE] = 1.0
    c2[COL_ASRC] = c_src
    c2[130] = c_dst
    c2b = np.broadcast_to(c2, (128, 131)).copy()

    ident = np.eye(128, dtype=np.float32)
    iota = np.broadcast_to(np.arange(128, dtype=np.float32), (128, 128)).copy()

    # edges + self loops, sorted by (core, block, src-half)
    src = np.concatenate([edge_index[0], np.arange(N, dtype=np.int64)]).astype(np.int64)
    dst = np.concatenate([edge_index[1], np.arange(N, dtype=np.int64)]).astype(np.int64)
    core = dst // SHARD
    local = dst - core * SHARD
    blk = local // 128
    half = (src >= HALF).astype(np.int64)
    key = ((core * NBLK + blk) * 2 + half)
    order = np.argsort(key, kind="stable")
    src, dst, key = src[order], dst[order], key[order]
    counts = np.bincount(key, minlength=NCORES * NBLK * 2).reshape(NCORES, NBLK, 2)
    tiles = -(-counts // 128)                              # ceil
    tlo = tuple(int(t) for t in tiles[:, :, 0].max(axis=0))
    thi = tuple(int(t) for t in tiles[:, :, 1].max(axis=0))
    CB = sum(tlo) + sum(thi)

    # per-core slot tables in global column-block (cb) order
    feat_idx = np.zeros((NCORES, CB * 128), np.int16)
    adst_idx = np.zeros((NCORES, CB * 128), np.int16)
    dloc = np.full((NCORES, 128, CB), 128.0, np.float32)

    starts = np.zeros(NCORES * NBLK * 2 + 1, np.int64)
    starts[1:] = np.cumsum(counts.reshape(-1))

    # cb offset of each (block, half) segment, same for every core
    seg_off = {}
    cb0 = 0
    for g0 in range(0, NBLK, GBLK):
        blocks = list(range(g0, min(NBLK, g0 + GBLK)))
        off = cb0
        for b in blocks:
            seg_off[(b, 0)] = off
            off += tlo[b]
        for b in blocks:
            seg_off[(b, 1)] = off
            off += thi[b]
        cb0 = off
    assert cb0 == CB

    for c in range(NCORES):
        for b in range(NBLK):
            for hf in range(2):
                gi = (c * NBLK + b) * 2 + hf
                s, e = starts[gi], starts[gi + 1]
                n = int(e - s)
                if n == 0:
                    continue
                off = seg_off[(b, hf)]
                k = np.arange(n) + off * 128
                fi = (src[s:e] - hf * HALF).astype(np.int16)
                feat_idx[c, k] = fi
                ai = (dst[s:e] - c * SHARD).astype(np.int16)
                adst_idx[c, k] = ai
                p = k % 128
                t = k // 128
                dloc[c, p, t] = (dst[s:e] - (c * SHARD + b * 128)).astype(np.float32)

    in_maps = []
    for c in range(NCORES):
        xs = np.zeros((PAD_SHARD, D), np.float32)
        xs[0:SHARD] = x[c * SHARD : (c + 1) * SHARD]
        in_maps.append(
            {
                "x_shard": xs,
                "wext": wext,
                "c2b": c2b,
                "ident": ident,
                "iota": iota,
                "feat_idx": _wrap_idx(feat_idx[c]),
                "adst_idx": _wrap_idx(adst_idx[c]),
                "dloc": np.ascontiguousarray(dloc[c]),
            }
        )
    return tlo, thi, in_maps


_PROGRAM_CACHE = {}


def kernel(x, edge_index, edge_attr, h, batch, ln_gamma, ln_beta, W, att_src,
           att_dst, bias):
    x = np.asarray(x, dtype=np.float32)
    edge_index = np.asarray(edge_index)
    h = np.asarray(h)
    ln_gamma = np.asarray(ln_gamma, dtype=np.float32)
    ln_beta = np.asarray(ln_beta, dtype=np.float32)
    W = np.asarray(W, dtype=np.float32)
    att_src = np.asarray(att_src, dtype=np.float32)
    att_dst = np.asarray(att_dst, dtype=np.float32)
    bias = np.asarray(bias, dtype=np.float32)

    tlo, thi, in_maps = _host_prep(
        x, edge_index, ln_gamma, ln_beta, W, att_src, att_dst, bias
    )
    key = (tlo, thi)
    if key not in _PROGRAM_CACHE:
        _PROGRAM_CACHE[key] = _build_program(tlo, thi)
    nc = _PROGRAM_CACHE[key]

    res = run_bass_kernel_spmd(nc, in_maps, core_ids=list(range(NCORES)))
    out = np.concatenate([res.results[c]["out_shard"] for c in range(NCORES)], axis=0)
    return out, h



# revision 3
# speedup vs baseline: 1.6547x; 1.6547x over previous
"""GAT layer (LayerNorm -> GATConv(heads=1) -> residual ReLU) on 8 trn2 NeuronCores.

Sharding: destination-node parallel. Each core owns N/8 nodes: it computes the
node transform for its shard, shards are AllGathered (bf16, 256 B rows), and
each core processes the edges whose destination falls in its shard.

Per-edge source features are fetched with a 256 B transposed dma_gather
(gfT[f, e] = xp[src_e, f], bf16) -- 1 SWDGE descriptor per edge instead of the
3+1 of the 768B+256B row scheme. A per-column-block "un-transpose" matmul
(lhsT=gfT, rhs=[I | att_src]) recovers [gf | a_src] in PSUM, so a_src needs no
extra gather. a_dst is looked up on the tensor engine: a host-precomputed
one-hot ohT[r, e] = (dst_local_e == r) (bf16, streamed sequentially) times the
block's a_dst column. ee = exp(leakyrelu(a_src + a_dst)) on DVE/ACT, and the
scatter-add is the usual one-hot matmul sw[e, r] = (iota_r == dstlocal_e)*ee_e
with a ones column for the softmax denominator.
"""

import ml_dtypes
import numpy as np

import concourse.bacc as bacc
import concourse.mybir as mybir
import concourse.tile as tile
from concourse.bass_utils import run_bass_kernel_spmd

F32 = mybir.dt.float32
BF16 = mybir.dt.bfloat16
I16 = mybir.dt.int16
AX = mybir.AxisListType
OP = mybir.AluOpType
AF = mybir.ActivationFunctionType
NPBF = ml_dtypes.bfloat16

N = 50000
D = 128
E = 600000
NCORES = 8
SHARD = N // NCORES            # 6250
NBLK = (SHARD + 127) // 128    # 49 dst blocks per core
PAD_SHARD = NBLK * 128         # 6272
LAST_ROWS = SHARD - (NBLK - 1) * 128  # 106
HALF = 32768                   # int16 index split point for the global table
NEG_SLOPE = 0.2
LN_EPS = 1e-5
GBLK = 5                       # dst blocks per gather group


def _group_layout(tlo, thi):
    """Group structure + per-group (cb_local -> dst block) maps."""
    groups = []
    cb0 = 0
    for g0 in range(0, NBLK, GBLK):
        blocks = list(range(g0, min(NBLK, g0 + GBLK)))
        nlo = sum(tlo[b] for b in blocks)
        nhi = sum(thi[b] for b in blocks)
        # per-block local cb indices: lo segments first, then hi segments
        per_block = {b: [] for b in blocks}
        off = 0
        for b in blocks:
            per_block[b].extend(range(off, off + tlo[b]))
            off += tlo[b]
        for b in blocks:
            per_block[b].extend(range(off, off + thi[b]))
            off += thi[b]
        groups.append((blocks, cb0, nlo, nhi, per_block))
        cb0 += nlo + nhi
    return groups, cb0


def _build_program(tlo, thi):
    nc = bacc.Bacc("TRN2", num_devices=NCORES, debug=False)

    groups, CB = _group_layout(tlo, thi)
    CBG_MAX = max(nlo + nhi for _, _, nlo, nhi, _ in groups)

    x_shard = nc.dram_tensor("x_shard", [PAD_SHARD, D], F32, kind="ExternalInput")
    wext = nc.dram_tensor("wext", [D, 129], F32, kind="ExternalInput")
    c2b = nc.dram_tensor("c2b", [128, 129], F32, kind="ExternalInput")
    ident = nc.dram_tensor("ident", [128, 128], F32, kind="ExternalInput")
    identvs = nc.dram_tensor("identvs", [128, 129], BF16, kind="ExternalInput")
    iota = nc.dram_tensor("iota", [128, 128], BF16, kind="ExternalInput")
    feat_idx = nc.dram_tensor("feat_idx", [128, CB * 8], I16, kind="ExternalInput")
    dloc = nc.dram_tensor("dloc", [128, CB], F32, kind="ExternalInput")
    ohT = nc.dram_tensor("ohT", [128, CB * 128], BF16, kind="ExternalInput")
    out_shard = nc.dram_tensor("out_shard", [SHARD, D], F32, kind="ExternalOutput")

    with tile.TileContext(nc) as tc:
        with (
            tc.tile_pool(name="dram", bufs=1, space="DRAM") as dram,
            tc.tile_pool(name="consts", bufs=1) as cpool,
            tc.tile_pool(name="xres", bufs=1) as xpool,
        ):
            xp_shard = dram.tile([SHARD, D], BF16)
            xp_full = dram.tile([N, D], BF16, addr_space="Shared")

            ident_sb = cpool.tile([128, 128], F32)
            nc.sync.dma_start(ident_sb[:], ident[:, :])
            identvs_sb = cpool.tile([128, 129], BF16)
            nc.sync.dma_start(identvs_sb[:], identvs[:, :])
            iota_sb = cpool.tile([128, 128], BF16)
            nc.sync.dma_start(iota_sb[:], iota[:, :])
            wext_sb = cpool.tile([D, 129], F32)
            nc.sync.dma_start(wext_sb[:], wext[:, :])
            c2b_sb = cpool.tile([128, 129], F32)
            nc.sync.dma_start(c2b_sb[:], c2b[:, :])
            eps_sb = cpool.tile([128, 1], F32)
            nc.vector.memset(eps_sb[:], LN_EPS)
            ones_sb = cpool.tile([128, CBG_MAX], BF16)
            nc.vector.memset(ones_sb[:], 1.0)
            fidx_sb = cpool.tile([128, CB * 8], I16)
            nc.sync.dma_start(fidx_sb[:], feat_idx[:, :])
            dl_sb = cpool.tile([128, CB], F32)
            nc.sync.dma_start(dl_sb[:], dloc[:, :])
            adst_sb = cpool.tile([128, NBLK], BF16)

            x_tiles = []
            for i in range(NBLK):
                xt = xpool.tile([128, D], F32, tag=f"xres{i}")
                nc.sync.dma_start(xt[:], x_shard[i * 128 : (i + 1) * 128, :])
                x_tiles.append(xt)

            # ---------------- Phase A: node transform on own shard ---------
            with (
                tc.tile_pool(name="a_small", bufs=8) as spool,
                tc.tile_pool(name="a_sq", bufs=2) as sqpool,
                tc.tile_pool(name="a_xnp", bufs=3) as xnppool,
                tc.tile_pool(name="a_xnpT", bufs=3) as xnptpool,
                tc.tile_pool(name="a_xpe", bufs=3) as xpepool,
                tc.tile_pool(name="a_ps_t", bufs=2, space="PSUM") as psa,
                tc.tile_pool(name="a_ps_m", bufs=2, space="PSUM") as psb,
            ):
                for i in range(NBLK):
                    xt = x_tiles[i]
                    rows = 128 if i < NBLK - 1 else LAST_ROWS
                    sumx = spool.tile([128, 1], F32, tag="sumx")
                    nc.vector.tensor_reduce(sumx[:], xt[:], AX.X, OP.add)
                    sqj = sqpool.tile([128, D], F32)
                    ssq = spool.tile([128, 1], F32, tag="ssq")
                    nc.scalar.activation(sqj[:], xt[:], AF.Square, accum_out=ssq[:])
                    mu = spool.tile([128, 1], F32, tag="mu")
                    nc.vector.tensor_scalar(mu[:], sumx[:], 1.0 / D, None, OP.mult)
                    m2 = spool.tile([128, 1], F32, tag="m2")
                    nc.vector.tensor_tensor(m2[:], mu[:], mu[:], OP.mult)
                    var = spool.tile([128, 1], F32, tag="var")
                    nc.vector.tensor_scalar(
                        var[:], ssq[:], 1.0 / D, m2[:, 0:1], OP.mult, OP.subtract
                    )
                    std = spool.tile([128, 1], F32, tag="std")
                    nc.scalar.activation(std[:], var[:], AF.Sqrt, bias=eps_sb[:, 0:1])
                    rstd = spool.tile([128, 1], F32, tag="rstd")
                    nc.vector.reciprocal(rstd[:], std[:])
                    xnp = xnppool.tile([128, D], F32)
                    nc.vector.tensor_scalar(
                        xnp[:], xt[:], mu[:, 0:1], rstd[:, 0:1], OP.subtract, OP.mult
                    )
                    pt = psa.tile([128, 128], F32, space="PSUM")
                    nc.tensor.transpose(pt[:], xnp[:], ident_sb[:])
                    xnpT = xnptpool.tile([128, 128], F32)
                    nc.scalar.copy(xnpT[:], pt[:])
                    pm = psb.tile([128, 129], F32, space="PSUM")
                    nc.tensor.matmul(
                        pm[:], lhsT=xnpT[:], rhs=wext_sb[:], start=True, stop=True
                    )
                    xpe = xpepool.tile([128, 129], BF16)
                    nc.vector.tensor_tensor(xpe[:], pm[:], c2b_sb[:], OP.add)
                    nc.sync.dma_start(
                        xp_shard[i * 128 : i * 128 + rows, :], xpe[:rows, 0:128]
                    )
                    nc.vector.tensor_copy(adst_sb[:, i : i + 1], xpe[:, 128:129])

            nc.gpsimd.collective_compute(
                "AllGather",
                OP.bypass,
                replica_groups=[list(range(NCORES))],
                ins=[xp_shard[:, :]],
                outs=[xp_full[:, :]],
            )

            # ---------------- Phase B: edge aggregation --------------------
            with (
                tc.tile_pool(name="b_g", bufs=2) as gpool,
                tc.tile_pool(name="b_oh", bufs=2) as opool,
                tc.tile_pool(name="b_f", bufs=2) as fpool,
                tc.tile_pool(name="b_sw", bufs=4) as swpool,
                tc.tile_pool(name="b_e", bufs=3) as epool,
                tc.tile_pool(name="b_o", bufs=4) as outpool,
                tc.tile_pool(name="b_ps1", bufs=3, space="PSUM") as ps1pool,
                tc.tile_pool(name="b_pso", bufs=3, space="PSUM") as psopool,
                tc.tile_pool(name="b_psa", bufs=2, space="PSUM") as psapool,
            ):
                for blocks, cb0, nlo, nhi, per_block in groups:
                    cbg = nlo + nhi
                    gfT = gpool.tile([128, 1, CBG_MAX * 128], BF16, tag="gfT")
                    if nlo:
                        nc.gpsimd.dma_gather(
                            out_ap=gfT[:, :, 0 : nlo * 128],
                            in_ap=xp_full[0:HALF, :],
                            idxs_ap=fidx_sb[:, cb0 * 8 : (cb0 + nlo) * 8],
                            num_idxs=nlo * 128,
                            num_idxs_reg=nlo * 128,
                            elem_size=D,
                            transpose=True,
                            single_packet=False,
                        )
                    if nhi:
                        nc.gpsimd.dma_gather(
                            out_ap=gfT[:, :, nlo * 128 : cbg * 128],
                            in_ap=xp_full[HALF:N, :],
                            idxs_ap=fidx_sb[:, (cb0 + nlo) * 8 : (cb0 + cbg) * 8],
                            num_idxs=nhi * 128,
                            num_idxs_reg=nhi * 128,
                            elem_size=D,
                            transpose=True,
                            single_packet=False,
                        )
                    ohTg = opool.tile([128, CBG_MAX * 128], BF16, tag="ohTg")
                    nc.sync.dma_start(
                        ohTg[:, 0 : cbg * 128], ohT[:, cb0 * 128 : (cb0 + cbg) * 128]
                    )
                    gfsb = fpool.tile([128, CBG_MAX, 130], BF16, tag="gfsb")
                    ps_adst = psapool.tile([128, CBG_MAX], F32, space="PSUM")

                    # per-cb: un-transpose + a_src column; a_dst one-hot lookup
                    cb_block = {}
                    for b, js in per_block.items():
                        for j in js:
                            cb_block[j] = b
                    for j in range(cbg):
                        b = cb_block[j]
                        ps1 = ps1pool.tile([128, 129], F32, space="PSUM")
                        nc.tensor.matmul(
                            ps1[:],
                            lhsT=gfT[:, 0, j * 128 : (j + 1) * 128],
                            rhs=identvs_sb[:],
                            start=True,
                            stop=True,
                        )
                        nc.tensor.matmul(
                            ps_adst[:, j : j + 1],
                            lhsT=ohTg[:, j * 128 : (j + 1) * 128],
                            rhs=adst_sb[:, b : b + 1],
                            start=True,
                            stop=True,
                        )
                        if j % 2 == 0:
                            nc.scalar.copy(gfsb[:, j, 0:129], ps1[:])
                        else:
                            nc.vector.tensor_copy(gfsb[:, j, 0:129], ps1[:])
                    # ones column for the softmax denominator
                    nc.vector.tensor_copy(gfsb[:, 0:cbg, 129], ones_sb[:, 0:cbg])
                    # ee = exp(leakyrelu(a_src + a_dst)) for the whole group
                    adst_bg = epool.tile([128, CBG_MAX], BF16, tag="adst_bg")
                    nc.vector.tensor_copy(adst_bg[:, 0:cbg], ps_adst[:, 0:cbg])
                    e1 = epool.tile([128, CBG_MAX], BF16, tag="e1")
                    nc.vector.tensor_tensor(
                        e1[:, 0:cbg], gfsb[:, 0:cbg, 128], adst_bg[:, 0:cbg], OP.add
                    )
                    e2 = epool.tile([128, CBG_MAX], BF16, tag="e2")
                    nc.vector.tensor_scalar(
                        e2[:, 0:cbg], e1[:, 0:cbg], NEG_SLOPE, None, OP.mult
                    )
                    e3 = epool.tile([128, CBG_MAX], BF16, tag="e3")
                    nc.vector.tensor_tensor(
                        e3[:, 0:cbg], e2[:, 0:cbg], e1[:, 0:cbg], OP.max
                    )
                    ee = epool.tile([128, CBG_MAX], F32, tag="ee")
                    nc.scalar.activation(ee[:, 0:cbg], e3[:, 0:cbg], AF.Exp)

                    # per-block one-hot scatter matmuls
                    for b in blocks:
                        js = per_block[b]
                        rows = 128 if b < NBLK - 1 else LAST_ROWS
                        ps = psopool.tile([128, 130], F32, space="PSUM")
                        for k, j in enumerate(js):
                            sw = swpool.tile([128, 128], BF16)
                            nc.vector.tensor_scalar(
                                sw[:],
                                iota_sb[:],
                                dl_sb[:, cb0 + j : cb0 + j + 1],
                                ee[:, j : j + 1],
                                OP.is_equal,
                                OP.mult,
                            )
                            nc.tensor.matmul(
                                ps[:, :],
                                lhsT=sw[:],
                                rhs=gfsb[:, j, 0:130],
                                start=(k == 0),
                                stop=(k == len(js) - 1),
                            )
                        recip = epool.tile([128, 1], F32, tag="recip")
                        nc.vector.reciprocal(recip[:], ps[:, 129:130])
                        scaled = outpool.tile([128, D], F32, tag="scaled")
                        nc.scalar.activation(
                            scaled[:], ps[:, 0:D], AF.Copy, scale=recip[:, 0:1]
                        )
                        resid = outpool.tile([128, D], F32, tag="resid")
                        nc.vector.tensor_tensor(
                            resid[:], scaled[:], x_tiles[b][:], OP.add
                        )
                        outt = outpool.tile([128, D], F32, tag="outt")
                        nc.scalar.activation(outt[:], resid[:], AF.Relu)
                        nc.sync.dma_start(
                            out_shard[b * 128 : b * 128 + rows, :], outt[:rows, :]
                        )

    nc.compile()
    return nc


def _wrap_idx(idx):
    """int16 index list -> dma_gather SBUF layout [128, len/16]:
    index i lives at partitions {16g + i%16: g in 0..7}, column i//16."""
    L = len(idx)
    assert L % 16 == 0
    w = idx.reshape(L // 16, 16).T.astype(np.int16)      # [16, L/16]
    return np.tile(w, (8, 1))                            # [128, L/16]


def _host_prep(x, edge_index, ln_gamma, ln_beta, W, att_src, att_dst, bias):
    """Fold parameters and bucket edges by destination block. Numpy only."""
    Wt = W.T.astype(np.float64)
    G = ln_gamma.astype(np.float64)[:, None] * Wt          # [D, D]
    crow = ln_beta.astype(np.float64) @ Wt                 # [D]
    v_dst = G @ att_dst.astype(np.float64)
    c_dst = float(crow @ att_dst.astype(np.float64))
    # gathered rows hold xp + crow + bias; a_src = row @ att_src needs a
    # -bias@att_src correction, folded into the a_dst column instead
    c_corr = -float(bias.astype(np.float64) @ att_src.astype(np.float64))

    wext = np.zeros((D, 129), np.float32)
    wext[:, 0:D] = G.astype(np.float32)
    wext[:, 128] = v_dst.astype(np.float32)
    c2 = np.zeros((129,), np.float32)
    c2[0:D] = (crow + bias.astype(np.float64)).astype(np.float32)
    c2[128] = c_dst + c_corr
    c2b = np.broadcast_to(c2, (128, 129)).copy()

    ident = np.eye(128, dtype=np.float32)
    identvs = np.zeros((128, 129), NPBF)
    identvs[:, 0:128] = np.eye(128, dtype=np.float32).astype(NPBF)
    identvs[:, 128] = att_src.astype(np.float32).astype(NPBF)
    iota = np.broadcast_to(np.arange(128, dtype=np.float32), (128, 128)).astype(NPBF).copy()

    # edges + self loops, sorted by (core, block, src-half)
    src = np.concatenate([edge_index[0], np.arange(N, dtype=np.int64)]).astype(np.int64)
    dst = np.concatenate([edge_index[1], np.arange(N, dtype=np.int64)]).astype(np.int64)
    core = dst // SHARD
    local = dst - core * SHARD
    blk = local // 128
    half = (src >= HALF).astype(np.int64)
    key = ((core * NBLK + blk) * 2 + half)
    order = np.argsort(key, kind="stable")
    src, dst, key = src[order], dst[order], key[order]
    counts = np.bincount(key, minlength=NCORES * NBLK * 2).reshape(NCORES, NBLK, 2)
    tiles = -(-counts // 128)                              # ceil
    tlo = tuple(int(t) for t in tiles[:, :, 0].max(axis=0))
    thi = tuple(int(t) for t in tiles[:, :, 1].max(axis=0))
    groups, CB = _group_layout(tlo, thi)

    # per-core slot tables in global column-block (cb) order
    feat_idx = np.zeros((NCORES, CB * 128), np.int16)
    dloc = np.full((NCORES, 128, CB), 128.0, np.float32)
    ohT = np.zeros((NCORES, 128, CB * 128), NPBF)

    starts = np.zeros(NCORES * NBLK * 2 + 1, np.int64)
    starts[1:] = np.cumsum(counts.reshape(-1))

    # cb offset of each (block, half) segment, same for every core
    seg_off = {}
    for blocks, cb0, nlo, nhi, per_block in groups:
        off = cb0
        for b in blocks:
            seg_off[(b, 0)] = off
            off += tlo[b]
        for b in blocks:
            seg_off[(b, 1)] = off
            off += thi[b]

    for c in range(NCORES):
        for b in range(NBLK):
            for hf in range(2):
                gi = (c * NBLK + b) * 2 + hf
                s, e = starts[gi], starts[gi + 1]
                n = int(e - s)
                if n == 0:
                    continue
                off = seg_off[(b, hf)]
                k = np.arange(n) + off * 128
                fi = (src[s:e] - hf * HALF).astype(np.int16)
                feat_idx[c, k] = fi
                dl = (dst[s:e] - (c * SHARD + b * 128)).astype(np.int64)
                p = k % 128
                t = k // 128
                dloc[c, p, t] = dl.astype(np.float32)
                ohT[c, dl, k] = 1.0

    in_maps = []
    for c in range(NCORES):
        xs = np.zeros((PAD_SHARD, D), np.float32)
        xs[0:SHARD] = x[c * SHARD : (c + 1) * SHARD]
        in_maps.append(
            {
                "x_shard": xs,
                "wext": wext,
                "c2b": c2b,
                "ident": ident,
                "identvs": identvs,
                "iota": iota,
                "feat_idx": _wrap_idx(feat_idx[c]),
                "dloc": np.ascontiguousarray(dloc[c]),
                "ohT": np.ascontiguousarray(ohT[c]),
            }
        )
    return tlo, thi, in_maps


_PROGRAM_CACHE = {}


def kernel(x, edge_index, edge_attr, h, batch, ln_gamma, ln_beta, W, att_src,
           att_dst, bias):
    x = np.asarray(x, dtype=np.float32)
    edge_index = np.asarray(edge_index)
    h = np.asarray(h)
    ln_gamma = np.asarray(ln_gamma, dtype=np.float32)
    ln_beta = np.asarray(ln_beta, dtype=np.float32)
    W = np.asarray(W, dtype=np.float32)
    att_src = np.asarray(att_src, dtype=np.float32)
    att_dst = np.asarray(att_dst, dtype=np.float32)
    bias = np.asarray(bias, dtype=np.float32)

    tlo, thi, in_maps = _host_prep(
        x, edge_index, ln_gamma, ln_beta, W, att_src, att_dst, bias
    )
    key = (tlo, thi)
    if key not in _PROGRAM_CACHE:
        _PROGRAM_CACHE[key] = _build_program(tlo, thi)
    nc = _PROGRAM_CACHE[key]

    res = run_bass_kernel_spmd(nc, in_maps, core_ids=list(range(NCORES)))
    out = np.concatenate([res.results[c]["out_shard"] for c in range(NCORES)], axis=0)
    return out, h


# revision 5
# speedup vs baseline: 1.8640x; 1.1265x over previous
"""GAT layer (LayerNorm -> GATConv(heads=1) -> residual ReLU) on 8 trn2 NeuronCores.

Sharding: destination-node parallel. Each core owns N/8 nodes: it computes the
node transform for its shard, shards are AllGathered (bf16, 256 B rows), and
each core processes the edges whose destination falls in its shard.

Per-edge source records are fetched with 256 B non-transposed dma_gathers.
SWDGE descriptor generation (~8 ns/row) is the critical resource, so the
kernel (a) splits edges into own-shard / low / high segments -- own-shard
edges gather from the local table and overlap the AllGather, (b) keeps the
Vector engine off 2-port DVE modes during phase B (they lock GpSimd out of
SBUF and slow descgen): both one-hot operands are host-precomputed and
DMA-streamed, and the per-edge ee scaling runs on the Scalar engine.

The gathered row packs a_src INTO the feature vector: row[jmax] =
sum_f att_src[f]*xp[f] with jmax = argmax|att_src|; the aggregated feature
jmax is recovered post-scatter from the same linear identity. a_dst is looked
up on the tensor engine (ohT[r,e] one-hot times the block's a_dst column).
ee = exp(leakyrelu(a_src + a_dst)); the scatter-add is a one-hot matmul with
an ee column in the rhs producing the softmax denominator.
"""

import ml_dtypes
import numpy as np

import concourse.bacc as bacc
import concourse.mybir as mybir
import concourse.tile as tile
from concourse.bass_utils import run_bass_kernel_spmd

F32 = mybir.dt.float32
BF16 = mybir.dt.bfloat16
I16 = mybir.dt.int16
AX = mybir.AxisListType
OP = mybir.AluOpType
AF = mybir.ActivationFunctionType
NPBF = ml_dtypes.bfloat16

N = 50000
D = 128
E = 600000
NCORES = 8
SHARD = N // NCORES            # 6250
NBLK = (SHARD + 127) // 128    # 49 dst blocks per core
PAD_SHARD = NBLK * 128         # 6272
LAST_ROWS = SHARD - (NBLK - 1) * 128  # 106
HALF = 32768                   # int16 index split point for the global table
NEG_SLOPE = 0.2
LN_EPS = 1e-5
GBLK = 4                       # dst blocks per gather group


def _group_layout(t3):
    """t3: [NBLK][3] tile counts (own, lo, hi). Returns group list
    (blocks, cb0, nseg=(nown, nlo, nhi), per_block cb-lists) and CB."""
    groups = []
    cb0 = 0
    for g0 in range(0, NBLK, GBLK):
        blocks = list(range(g0, min(NBLK, g0 + GBLK)))
        nseg = [sum(t3[b][s] for b in blocks) for s in range(3)]
        per_block = {b: [] for b in blocks}
        off = 0
        for s in range(3):
            for b in blocks:
                per_block[b].extend(range(off, off + t3[b][s]))
                off += t3[b][s]
        groups.append((blocks, cb0, tuple(nseg), per_block))
        cb0 += sum(nseg)
    return groups, cb0


def _build_program(t3, jmax, inv_ajmax):
    nc = bacc.Bacc("TRN2", num_devices=NCORES, debug=False)

    groups, CB = _group_layout(t3)
    CBG_MAX = max(sum(nseg) for _, _, nseg, _ in groups)

    x_shard = nc.dram_tensor("x_shard", [PAD_SHARD, D], F32, kind="ExternalInput")
    wext = nc.dram_tensor("wext", [D, 129], F32, kind="ExternalInput")
    c2b = nc.dram_tensor("c2b", [128, 129], F32, kind="ExternalInput")
    ident = nc.dram_tensor("ident", [128, 128], F32, kind="ExternalInput")
    attb = nc.dram_tensor("attb", [128, 128], F32, kind="ExternalInput")
    feat_idx = nc.dram_tensor("feat_idx", [128, CB * 8], I16, kind="ExternalInput")
    oh_t = nc.dram_tensor("oh_t", [128, CB * 128], BF16, kind="ExternalInput")
    ohT_t = nc.dram_tensor("ohT_t", [128, CB * 128], BF16, kind="ExternalInput")
    out_shard = nc.dram_tensor("out_shard", [SHARD, D], F32, kind="ExternalOutput")

    with tile.TileContext(nc) as tc:
        with (
            tc.tile_pool(name="dram", bufs=1, space="DRAM") as dram,
            tc.tile_pool(name="consts", bufs=1) as cpool,
            tc.tile_pool(name="xres", bufs=1) as xpool,
        ):
            xp_shard = dram.tile([SHARD, D], BF16)
            xp_full = dram.tile([N, D], BF16, addr_space="Shared")

            ident_sb = cpool.tile([128, 128], F32)
            nc.sync.dma_start(ident_sb[:], ident[:, :])
            attb_sb = cpool.tile([128, 128], F32)
            nc.sync.dma_start(attb_sb[:], attb[:, :])
            wext_sb = cpool.tile([D, 129], F32)
            nc.sync.dma_start(wext_sb[:], wext[:, :])
            c2b_sb = cpool.tile([128, 129], F32)
            nc.sync.dma_start(c2b_sb[:], c2b[:, :])
            eps_sb = cpool.tile([128, 1], F32)
            nc.vector.memset(eps_sb[:], LN_EPS)
            fidx_sb = cpool.tile([128, CB * 8], I16)
            nc.sync.dma_start(fidx_sb[:], feat_idx[:, :])
            adst_sb = cpool.tile([128, NBLK], BF16)

            x_tiles = []
            for i in range(NBLK):
                xt = xpool.tile([128, D], F32, tag=f"xres{i}")
                nc.sync.dma_start(xt[:], x_shard[i * 128 : (i + 1) * 128, :])
                x_tiles.append(xt)

            # ---------------- Phase A: node transform on own shard ---------
            with (
                tc.tile_pool(name="a_small", bufs=8) as spool,
                tc.tile_pool(name="a_sq", bufs=2) as sqpool,
                tc.tile_pool(name="a_xnp", bufs=3) as xnppool,
                tc.tile_pool(name="a_xnpT", bufs=3) as xnptpool,
                tc.tile_pool(name="a_xpe", bufs=3) as xpepool,
                tc.tile_pool(name="a_ps_t", bufs=2, space="PSUM") as psa,
                tc.tile_pool(name="a_ps_m", bufs=2, space="PSUM") as psb,
            ):
                for i in range(NBLK):
                    xt = x_tiles[i]
                    rows = 128 if i < NBLK - 1 else LAST_ROWS
                    sumx = spool.tile([128, 1], F32, tag="sumx")
                    nc.vector.tensor_reduce(sumx[:], xt[:], AX.X, OP.add)
                    sqj = sqpool.tile([128, D], F32)
                    ssq = spool.tile([128, 1], F32, tag="ssq")
                    nc.scalar.activation(sqj[:], xt[:], AF.Square, accum_out=ssq[:])
                    mu = spool.tile([128, 1], F32, tag="mu")
                    nc.vector.tensor_scalar(mu[:], sumx[:], 1.0 / D, None, OP.mult)
                    m2 = spool.tile([128, 1], F32, tag="m2")
                    nc.vector.tensor_tensor(m2[:], mu[:], mu[:], OP.mult)
                    var = spool.tile([128, 1], F32, tag="var")
                    nc.vector.tensor_scalar(
                        var[:], ssq[:], 1.0 / D, m2[:, 0:1], OP.mult, OP.subtract
                    )
                    std = spool.tile([128, 1], F32, tag="std")
                    nc.scalar.activation(std[:], var[:], AF.Sqrt, bias=eps_sb[:, 0:1])
                    rstd = spool.tile([128, 1], F32, tag="rstd")
                    nc.vector.reciprocal(rstd[:], std[:])
                    xnp = xnppool.tile([128, D], F32)
                    nc.vector.tensor_scalar(
                        xnp[:], xt[:], mu[:, 0:1], rstd[:, 0:1], OP.subtract, OP.mult
                    )
                    pt = psa.tile([128, 128], F32, space="PSUM")
                    nc.tensor.transpose(pt[:], xnp[:], ident_sb[:])
                    xnpT = xnptpool.tile([128, 128], F32)
                    nc.scalar.copy(xnpT[:], pt[:])
                    pm = psb.tile([128, 129], F32, space="PSUM")
                    nc.tensor.matmul(
                        pm[:], lhsT=xnpT[:], rhs=wext_sb[:], start=True, stop=True
                    )
                    xpe = xpepool.tile([128, 129], BF16)
                    nc.vector.tensor_tensor(xpe[:], pm[:], c2b_sb[:], OP.add)
                    nc.sync.dma_start(
                        xp_shard[i * 128 : i * 128 + rows, :], xpe[:rows, 0:128]
                    )
                    nc.vector.tensor_copy(adst_sb[:, i : i + 1], xpe[:, 128:129])

            nc.gpsimd.collective_compute(
                "AllGather",
                OP.bypass,
                replica_groups=[list(range(NCORES))],
                ins=[xp_shard[:, :]],
                outs=[xp_full[:, :]],
            )

            # ---------------- Phase B: edge aggregation --------------------
            with (
                tc.tile_pool(name="b_g", bufs=2) as gpool,
                tc.tile_pool(name="b_oh", bufs=2) as opool,
                tc.tile_pool(name="b_ohT", bufs=2) as otpool,
                tc.tile_pool(name="b_f", bufs=2) as fpool,
                tc.tile_pool(name="b_e", bufs=3) as epool,
                tc.tile_pool(name="b_o", bufs=4) as outpool,
                tc.tile_pool(name="b_pso", bufs=4, space="PSUM") as psopool,
                tc.tile_pool(name="b_psa", bufs=2, space="PSUM") as psapool,
            ):
                for blocks, cb0, (nown, nlo, nhi), per_block in groups:
                    cbg = nown + nlo + nhi
                    gn = len(blocks)
                    g0 = blocks[0]
                    T = gpool.tile([128, CBG_MAX, D], BF16, tag="T")
                    if nown:
                        nc.gpsimd.dma_gather(
                            out_ap=T[:, 0:nown, :],
                            in_ap=xp_shard[:, :],
                            idxs_ap=fidx_sb[:, cb0 * 8 : (cb0 + nown) * 8],
                            num_idxs=nown * 128,
                            num_idxs_reg=nown * 128,
                            elem_size=D,
                            single_packet=False,
                        )
                    if nlo:
                        nc.gpsimd.dma_gather(
                            out_ap=T[:, nown : nown + nlo, :],
                            in_ap=xp_full[0:HALF, :],
                            idxs_ap=fidx_sb[:, (cb0 + nown) * 8 : (cb0 + nown + nlo) * 8],
                            num_idxs=nlo * 128,
                            num_idxs_reg=nlo * 128,
                            elem_size=D,
                            single_packet=False,
                        )
                    if nhi:
                        nc.gpsimd.dma_gather(
                            out_ap=T[:, nown + nlo : cbg, :],
                            in_ap=xp_full[HALF:N, :],
                            idxs_ap=fidx_sb[:, (cb0 + nown + nlo) * 8 : (cb0 + cbg) * 8],
                            num_idxs=nhi * 128,
                            num_idxs_reg=nhi * 128,
                            elem_size=D,
                            single_packet=False,
                        )
                    ohg = opool.tile([128, CBG_MAX * 128], BF16, tag="ohg")
                    nc.sync.dma_start(
                        ohg[:, 0 : cbg * 128], oh_t[:, cb0 * 128 : (cb0 + cbg) * 128]
                    )
                    ohTg = otpool.tile([128, CBG_MAX * 128], BF16, tag="ohTg")
                    nc.sync.dma_start(
                        ohTg[:, 0 : cbg * 128], ohT_t[:, cb0 * 128 : (cb0 + cbg) * 128]
                    )
                    # gate: group's a_dst columns, data-dependent on the own-
                    # shard gather so a_dst matmuls can't hoist into phase A
                    tz = epool.tile([128, 1], F32, tag="tz")
                    nc.vector.tensor_scalar(tz[:], T[:, 0, 0:1], 0.0, None, OP.mult)
                    adst_g = epool.tile([128, GBLK], BF16, tag="adst_g")
                    nc.vector.tensor_scalar(
                        adst_g[:, 0:gn], adst_sb[:, g0 : g0 + gn], tz[:, 0:1],
                        None, OP.add,
                    )

                    cb_block = {}
                    for b, js in per_block.items():
                        for j in js:
                            cb_block[j] = b
                    ps_adst = psapool.tile([128, CBG_MAX], F32, space="PSUM")
                    for j in range(cbg):
                        k = cb_block[j] - g0
                        nc.tensor.matmul(
                            ps_adst[:, j : j + 1],
                            lhsT=ohTg[:, j * 128 : (j + 1) * 128],
                            rhs=adst_g[:, k : k + 1],
                            start=True,
                            stop=True,
                        )
                    # ee = exp(leakyrelu(a_src + a_dst)) for the whole group
                    adst_bg = epool.tile([128, CBG_MAX], BF16, tag="adst_bg")
                    nc.vector.tensor_copy(adst_bg[:, 0:cbg], ps_adst[:, 0:cbg])
                    e1 = epool.tile([128, CBG_MAX], BF16, tag="e1")
                    nc.vector.tensor_tensor(
                        e1[:, 0:cbg], T[:, 0:cbg, jmax], adst_bg[:, 0:cbg], OP.add
                    )
                    e2 = epool.tile([128, CBG_MAX], BF16, tag="e2")
                    nc.vector.tensor_scalar(
                        e2[:, 0:cbg], e1[:, 0:cbg], NEG_SLOPE, None, OP.mult
                    )
                    e3 = epool.tile([128, CBG_MAX], BF16, tag="e3")
                    nc.vector.tensor_tensor(
                        e3[:, 0:cbg], e2[:, 0:cbg], e1[:, 0:cbg], OP.max
                    )
                    ee = epool.tile([128, CBG_MAX], F32, tag="ee")
                    nc.scalar.activation(ee[:, 0:cbg], e3[:, 0:cbg], AF.Exp)

                    # rhs rows scaled by ee (Scalar engine; keeps DVE off
                    # 2-port modes during descgen), with an ee denom column
                    T2 = fpool.tile([128, CBG_MAX, D + 1], BF16, tag="T2")
                    for j in range(cbg):
                        nc.scalar.activation(
                            T2[:, j, 0:D], T[:, j, :], AF.Copy,
                            scale=ee[:, j : j + 1],
                        )
                    nc.vector.tensor_copy(T2[:, 0:cbg, D], ee[:, 0:cbg])

                    # per-block one-hot scatter matmuls
                    for b in blocks:
                        js = per_block[b]
                        rows = 128 if b < NBLK - 1 else LAST_ROWS
                        ps = psopool.tile([128, D + 1], F32, space="PSUM")
                        for k, j in enumerate(js):
                            nc.tensor.matmul(
                                ps[:, :],
                                lhsT=ohg[:, j * 128 : (j + 1) * 128],
                                rhs=T2[:, j, 0 : D + 1],
                                start=(k == 0),
                                stop=(k == len(js) - 1),
                            )
                        recip = epool.tile([128, 1], F32, tag="recip")
                        nc.vector.reciprocal(recip[:], ps[:, D : D + 1])
                        scaled = outpool.tile([128, D], F32, tag="scaled")
                        nc.scalar.activation(
                            scaled[:], ps[:, 0:D], AF.Copy, scale=recip[:, 0:1]
                        )
                        # recover feature jmax:
                        # out[jmax] = (S_a - sum_f a'_f out_f) / a_jmax
                        q = outpool.tile([128, D], F32, tag="q")
                        nc.vector.tensor_tensor(q[:], scaled[:], attb_sb[:], OP.mult)
                        qs = epool.tile([128, 1], F32, tag="qs")
                        nc.vector.tensor_reduce(qs[:], q[:], AX.X, OP.add)
                        numer = epool.tile([128, 1], F32, tag="numer")
                        nc.vector.tensor_tensor(
                            numer[:], scaled[:, jmax : jmax + 1], qs[:], OP.subtract
                        )
                        nc.vector.tensor_scalar(
                            scaled[:, jmax : jmax + 1], numer[:], inv_ajmax,
                            None, OP.mult,
                        )
                        resid = outpool.tile([128, D], F32, tag="resid")
                        nc.vector.tensor_tensor(
                            resid[:], scaled[:], x_tiles[b][:], OP.add
                        )
                        outt = outpool.tile([128, D], F32, tag="outt")
                        nc.scalar.activation(outt[:], resid[:], AF.Relu)
                        nc.sync.dma_start(
                            out_shard[b * 128 : b * 128 + rows, :], outt[:rows, :]
                        )

    nc.compile()
    return nc


def _wrap_idx(idx):
    """int16 index list -> dma_gather SBUF layout [128, len/16]:
    index i lives at partitions {16g + i%16: g in 0..7}, column i//16."""
    L = len(idx)
    assert L % 16 == 0
    w = idx.reshape(L // 16, 16).T.astype(np.int16)      # [16, L/16]
    return np.tile(w, (8, 1))                            # [128, L/16]


def _host_prep(x, edge_index, ln_gamma, ln_beta, W, att_src, att_dst, bias):
    """Fold parameters and bucket edges by destination block. Numpy only."""
    Wt = W.T.astype(np.float64)
    G = ln_gamma.astype(np.float64)[:, None] * Wt          # [D, D]
    crow = ln_beta.astype(np.float64) @ Wt                 # [D]
    a_src = att_src.astype(np.float64)
    v_src = G @ a_src
    v_dst = G @ att_dst.astype(np.float64)
    c_dst = float(crow @ att_dst.astype(np.float64))
    biasf = bias.astype(np.float64)

    jmax = int(np.argmax(np.abs(a_src)))
    inv_ajmax = float(1.0 / a_src[jmax])

    # table row = xp (= xn@G + crow + bias), except row[jmax] = sum_f a_f*xp_f
    wext = np.zeros((D, 129), np.float32)
    wext[:, 0:D] = G.astype(np.float32)
    wext[:, jmax] = v_src.astype(np.float32)
    wext[:, 128] = v_dst.astype(np.float32)
    c2 = np.zeros((129,), np.float32)
    c2[0:D] = (crow + biasf).astype(np.float32)
    c2[jmax] = float((crow + biasf) @ a_src)
    # e1 needs true a_src = row[jmax] - bias@att_src: fold into a_dst column
    c2[128] = c_dst - float(biasf @ a_src)
    c2b = np.broadcast_to(c2, (128, 129)).copy()

    ident = np.eye(128, dtype=np.float32)
    attb = np.broadcast_to(a_src.astype(np.float32), (128, 128)).copy()
    attb[:, jmax] = 0.0

    # edges + self loops, sorted by (core, block, own/lo/hi segment)
    src = np.concatenate([edge_index[0], np.arange(N, dtype=np.int64)]).astype(np.int64)
    dst = np.concatenate([edge_index[1], np.arange(N, dtype=np.int64)]).astype(np.int64)
    core = dst // SHARD
    local = dst - core * SHARD
    blk = local // 128
    own = (src // SHARD) == core
    seg = np.where(own, 0, 1 + (src >= HALF))
    key = (core * NBLK + blk) * 3 + seg
    order = np.argsort(key, kind="stable")
    src, dst, key, seg = src[order], dst[order], key[order], seg[order]
    core = core[order]
    counts = np.bincount(key, minlength=NCORES * NBLK * 3).reshape(NCORES, NBLK, 3)
    tiles = -(-counts // 128)                              # ceil
    t3 = tuple(
        tuple(int(tiles[:, b, s].max()) for s in range(3)) for b in range(NBLK)
    )
    groups, CB = _group_layout(t3)

    # per-core slot tables in global column-block (cb) order
    feat_idx = np.zeros((NCORES, CB * 128), np.int16)
    oh = np.zeros((NCORES, 128, CB * 128), NPBF)
    ohT = np.zeros((NCORES, 128, CB * 128), NPBF)

    starts = np.zeros(NCORES * NBLK * 3 + 1, np.int64)
    starts[1:] = np.cumsum(counts.reshape(-1))

    seg_off = {}
    for blocks, cb0, nseg, per_block in groups:
        off = cb0
        for s in range(3):
            for b in blocks:
                seg_off[(b, s)] = off
                off += t3[b][s]

    for c in range(NCORES):
        for b in range(NBLK):
            for s in range(3):
                gi = (c * NBLK + b) * 3 + s
                lo_, hi_ = starts[gi], starts[gi + 1]
                n = int(hi_ - lo_)
                if n == 0:
                    continue
                off = seg_off[(b, s)]
                k = np.arange(n) + off * 128
                base = c * SHARD if s == 0 else (0 if s == 1 else HALF)
                feat_idx[c, k] = (src[lo_:hi_] - base).astype(np.int16)
                dl = (dst[lo_:hi_] - (c * SHARD + b * 128)).astype(np.int64)
                p = k % 128
                t = k // 128
                oh[c, p, t * 128 + dl] = 1.0
                ohT[c, dl, k] = 1.0

    in_maps = []
    for c in range(NCORES):
        xs = np.zeros((PAD_SHARD, D), np.float32)
        xs[0:SHARD] = x[c * SHARD : (c + 1) * SHARD]
        in_maps.append(
            {
                "x_shard": xs,
                "wext": wext,
                "c2b": c2b,
                "ident": ident,
                "attb": attb,
                "feat_idx": _wrap_idx(feat_idx[c]),
                "oh_t": np.ascontiguousarray(oh[c]),
                "ohT_t": np.ascontiguousarray(ohT[c]),
            }
        )
    return t3, jmax, inv_ajmax, in_maps


_PROGRAM_CACHE = {}


def kernel(x, edge_index, edge_attr, h, batch, ln_gamma, ln_beta, W, att_src,
           att_dst, bias):
    x = np.asarray(x, dtype=np.float32)
    edge_index = np.asarray(edge_index)
    h = np.asarray(h)
    ln_gamma = np.asarray(ln_gamma, dtype=np.float32)
    ln_beta = np.asarray(ln_beta, dtype=np.float32)
    W = np.asarray(W, dtype=np.float32)
    att_src = np.asarray(att_src, dtype=np.float32)
    att_dst = np.asarray(att_dst, dtype=np.float32)
    bias = np.asarray(bias, dtype=np.float32)

    t3, jmax, inv_ajmax, in_maps = _host_prep(
        x, edge_index, ln_gamma, ln_beta, W, att_src, att_dst, bias
    )
    key = (t3, jmax)
    if key not in _PROGRAM_CACHE:
        _PROGRAM_CACHE[key] = _build_program(t3, jmax, inv_ajmax)
    nc = _PROGRAM_CACHE[key]

    res = run_bass_kernel_spmd(nc, in_maps, core_ids=list(range(NCORES)))
    out = np.concatenate([res.results[c]["out_shard"] for c in range(NCORES)], axis=0)
    return out, h


# revision 10
# speedup vs baseline: 2.2412x; 1.2024x over previous
"""GAT layer (LayerNorm -> GATConv(heads=1) -> residual ReLU) on 8 trn2 NeuronCores.

Sharding: destination-node parallel. Each core owns N/8 nodes: it computes the
node transform for its shard, shards are AllGathered (bf16, 256 B rows), and
each core processes the edges whose destination falls in its shard.

Per-edge source records are fetched with 256 B non-transposed dma_gathers.
SWDGE descriptor generation (~7.8 ns/row, Q7-bound) is the critical resource:
  * slots are packed at (group, segment) granularity -- columns may straddle
    two dst blocks; boundary columns simply get one extra accumulating
    matmul per extra block (one-hot tables are emitted per (column, block)).
  * own-shard edges (src in the core's own shard) gather from the local
    table and are issued before the AllGather, hiding their descgen.
  * the Vector engine avoids 2-port DVE modes during phase B (they lock
    GpSimd out of SBUF): one-hot operands are host-precomputed and streamed,
    per-edge ee scaling runs on the Scalar engine.

The gathered row packs a_src INTO the feature vector: row[jmax] =
sum_f att_src[f]*xp[f] with jmax = argmax|att_src|; the aggregated feature
jmax is recovered post-scatter from the same linear identity. a_dst is looked
up on the tensor engine (ohT[r,e] one-hot times the block's a_dst column).
ee = exp(leakyrelu(a_src + a_dst)); the scatter-add is a one-hot matmul with
an ee column in the rhs producing the softmax denominator.
"""

import ml_dtypes
import numpy as np

import concourse.bacc as bacc
import concourse.mybir as mybir
import concourse.tile as tile
from concourse.bass_utils import run_bass_kernel_spmd

F32 = mybir.dt.float32
BF16 = mybir.dt.bfloat16
I16 = mybir.dt.int16
AX = mybir.AxisListType
OP = mybir.AluOpType
AF = mybir.ActivationFunctionType
NPBF = ml_dtypes.bfloat16

N = 50000
D = 128
E = 600000
NCORES = 8
SHARD = N // NCORES            # 6250
NBLK = (SHARD + 127) // 128    # 49 dst blocks per core
PAD_SHARD = NBLK * 128         # 6272
LAST_ROWS = SHARD - (NBLK - 1) * 128  # 106
HALF = 32768
NEG_SLOPE = 0.2
LN_EPS = 1e-5
GBLK = 3                       # dst blocks per gather group
ABLK = 4                       # blocks per phase-A giga-iteration


def _build_program(layout, jmax, inv_ajmax):
    """layout: static per-group description (shared by all cores):
    list of dicts with keys
      blocks:   [b...]
      ncols:    {s: cols for seg s}            (s in 0=own,1=lo,2=hi)
      col0:     {s: global fidx column of seg s's first column}
      entries:  {s: [(b, j, eidx), ...]}       j = col within seg,
                                               eidx = global oh-entry index
      e0own / e0lohi: global entry range starts for the group's own / lo+hi
                      entry blocks (own entries contiguous; lo+hi contiguous)
      nent_own / nent_lohi
    plus layout_tot = dict(ncols_tot, nent_tot).
    """
    groups, tot = layout
    CBG_MAX = max(g["ncols"][1] + g["ncols"][2] for g in groups)
    OWN_MAX = max(g["ncols"][0] for g in groups)
    ENT_MAX = max(g["nent_own"] + g["nent_lohi"] for g in groups)
    CB = tot["ncols_tot"]
    NENT = tot["nent_tot"]

    nc = bacc.Bacc("TRN2", num_devices=NCORES, debug=False)

    x_shard = nc.dram_tensor("x_shard", [PAD_SHARD, D], F32, kind="ExternalInput")
    wext = nc.dram_tensor("wext", [D, 129], BF16, kind="ExternalInput")
    c2b = nc.dram_tensor("c2b", [128, 129], F32, kind="ExternalInput")
    ident = nc.dram_tensor("ident", [128, 128], F32, kind="ExternalInput")
    attb = nc.dram_tensor("attb", [128, 128], F32, kind="ExternalInput")
    feat_idx = nc.dram_tensor("feat_idx", [128, CB * 8], I16, kind="ExternalInput")
    oh_t = nc.dram_tensor("oh_t", [128, NENT * 128], BF16, kind="ExternalInput")
    ohT_t = nc.dram_tensor("ohT_t", [128, NENT * 128], BF16, kind="ExternalInput")
    out_shard = nc.dram_tensor("out_shard", [SHARD, D], F32, kind="ExternalOutput")

    with tile.TileContext(nc) as tc:
        with (
            tc.tile_pool(name="dram", bufs=1, space="DRAM") as dram,
            tc.tile_pool(name="consts", bufs=1) as cpool,
            tc.tile_pool(name="xres", bufs=1) as xpool,
            tc.tile_pool(name="ownt", bufs=1) as ownpool,
        ):
            xp_shard = dram.tile([SHARD, D], BF16)
            xp_full = dram.tile([N, D], BF16, addr_space="Shared")

            ident_sb = cpool.tile([128, 128], F32)
            nc.sync.dma_start(ident_sb[:], ident[:, :])
            attb_sb = cpool.tile([128, 128], F32)
            nc.sync.dma_start(attb_sb[:], attb[:, :])
            wext_sb = cpool.tile([D, 129], BF16)
            nc.sync.dma_start(wext_sb[:], wext[:, :])
            c2b_sb = cpool.tile([128, 129], F32)
            nc.sync.dma_start(c2b_sb[:], c2b[:, :])
            eps_sb = cpool.tile([128, 1], F32)
            nc.vector.memset(eps_sb[:], LN_EPS)
            fidx_sb = cpool.tile([128, CB * 8], I16)
            nc.sync.dma_start(fidx_sb[:], feat_idx[:, :])
            adst_sb = cpool.tile([128, NBLK], BF16)

            # x loaded 4 blocks per tile: partition p = row p of each block
            nga = (NBLK + ABLK - 1) // ABLK
            x_tiles = []
            for a in range(nga):
                nb = min(ABLK, NBLK - a * ABLK)
                xt = xpool.tile([128, ABLK, D], F32, tag=f"x4_{a}")
                nc.sync.dma_start(
                    xt[:, 0:nb, :],
                    x_shard[a * ABLK * 128 : (a * ABLK + nb) * 128, :].rearrange(
                        "(a p) b -> p a b", p=128
                    ),
                )
                x_tiles.append(xt)

            def xres(b):
                return x_tiles[b // ABLK][:, b % ABLK, :]

            # ---------------- Phase A: node transform on own shard ---------
            with (
                tc.tile_pool(name="a_small", bufs=6) as spool,
                tc.tile_pool(name="a_sq", bufs=3) as sqpool,
                tc.tile_pool(name="a_xnp", bufs=6) as xnppool,
                tc.tile_pool(name="a_xnpT", bufs=6) as xnptpool,
                tc.tile_pool(name="a_xpe", bufs=6) as xpepool,
                tc.tile_pool(name="a_ps_t", bufs=3, space="PSUM") as psa,
                tc.tile_pool(name="a_ps_m", bufs=3, space="PSUM") as psb,
            ):
                for a in range(nga):
                    nb = min(ABLK, NBLK - a * ABLK)
                    xt = x_tiles[a]
                    sum4 = spool.tile([128, ABLK], F32, tag="sum4")
                    nc.vector.tensor_reduce(sum4[:, 0:nb], xt[:, 0:nb, :], AX.X, OP.add)
                    sq4 = sqpool.tile([128, ABLK, D], F32)
                    nc.scalar.activation(sq4[:, 0:nb, :], xt[:, 0:nb, :], AF.Square)
                    ssq4 = spool.tile([128, ABLK], F32, tag="ssq4")
                    nc.vector.tensor_reduce(ssq4[:, 0:nb], sq4[:, 0:nb, :], AX.X, OP.add)
                    mu4 = spool.tile([128, ABLK], F32, tag="mu4")
                    nc.vector.tensor_scalar(mu4[:, 0:nb], sum4[:, 0:nb], 1.0 / D, None, OP.mult)
                    m24 = spool.tile([128, ABLK], F32, tag="m24")
                    nc.vector.tensor_tensor(m24[:, 0:nb], mu4[:, 0:nb], mu4[:, 0:nb], OP.mult)
                    var4 = spool.tile([128, ABLK], F32, tag="var4")
                    nc.vector.tensor_scalar(
                        var4[:, 0:nb], ssq4[:, 0:nb], 1.0 / D, None, OP.mult
                    )
                    nc.vector.tensor_tensor(
                        var4[:, 0:nb], var4[:, 0:nb], m24[:, 0:nb], OP.subtract
                    )
                    std4 = spool.tile([128, ABLK], F32, tag="std4")
                    nc.scalar.activation(std4[:, 0:nb], var4[:, 0:nb], AF.Sqrt, bias=eps_sb[:, 0:1])
                    rstd4 = spool.tile([128, ABLK], F32, tag="rstd4")
                    nc.vector.reciprocal(rstd4[:, 0:nb], std4[:, 0:nb])
                    for k in range(nb):
                        i = a * ABLK + k
                        rows = 128 if i < NBLK - 1 else LAST_ROWS
                        xnp = xnppool.tile([128, D], F32)
                        nc.vector.tensor_scalar(
                            xnp[:], xt[:, k, :], mu4[:, k : k + 1],
                            rstd4[:, k : k + 1], OP.subtract, OP.mult,
                        )
                        pt = psa.tile([128, 128], F32, space="PSUM")
                        nc.tensor.transpose(pt[:], xnp[:], ident_sb[:])
                        xnpT = xnptpool.tile([128, 128], BF16)
                        nc.scalar.copy(xnpT[:], pt[:])
                        pm = psb.tile([128, 129], F32, space="PSUM")
                        nc.tensor.matmul(
                            pm[:], lhsT=xnpT[:], rhs=wext_sb[:], start=True, stop=True
                        )
                        xpe = xpepool.tile([128, 129], BF16)
                        nc.vector.tensor_tensor(xpe[:], pm[:], c2b_sb[:], OP.add)
                        nc.sync.dma_start(
                            xp_shard[i * 128 : i * 128 + rows, :], xpe[:rows, 0:128]
                        )
                        nc.vector.tensor_copy(adst_sb[:, i : i + 1], xpe[:, 128:129])

            # own-shard gathers for every group, before/overlapping AllGather
            ownT = []
            for gi, g in enumerate(groups):
                ncol = g["ncols"][0]
                t = ownpool.tile([128, max(ncol, 1), D], BF16, tag=f"ownT{gi}")
                if ncol:
                    c0 = g["col0"][0]
                    nc.gpsimd.dma_gather(
                        out_ap=t[:, 0:ncol, :],
                        in_ap=xp_shard[:, :],
                        idxs_ap=fidx_sb[:, c0 * 8 : (c0 + ncol) * 8],
                        num_idxs=ncol * 128,
                        num_idxs_reg=ncol * 128,
                        elem_size=D,
                        single_packet=False,
                    )
                ownT.append(t)

            nc.gpsimd.collective_compute(
                "AllGather",
                OP.bypass,
                replica_groups=[list(range(NCORES))],
                ins=[xp_shard[:, :]],
                outs=[xp_full[:, :]],
            )

            # ---------------- Phase B: edge aggregation --------------------
            with (
                tc.tile_pool(name="b_g", bufs=3) as gpool,
                tc.tile_pool(name="b_oh", bufs=2) as opool,
                tc.tile_pool(name="b_ohT", bufs=2) as otpool,
                tc.tile_pool(name="b_f", bufs=2) as fpool,
                tc.tile_pool(name="b_e", bufs=3) as epool,
                tc.tile_pool(name="b_o", bufs=4) as outpool,
                tc.tile_pool(name="b_pso", bufs=4, space="PSUM") as psopool,
                tc.tile_pool(name="b_psa", bufs=2, space="PSUM") as psapool,
            ):
                for gi, g in enumerate(groups):
                    blocks = g["blocks"]
                    nown = g["ncols"][0]
                    nlo, nhi = g["ncols"][1], g["ncols"][2]
                    cbl = nlo + nhi          # lo+hi cols in T
                    cbg = nown + cbl         # total cols this group
                    T = gpool.tile([128, CBG_MAX, D], BF16, tag="T")
                    if nlo:
                        c0 = g["col0"][1]
                        nc.gpsimd.dma_gather(
                            out_ap=T[:, 0:nlo, :],
                            in_ap=xp_full[0:HALF, :],
                            idxs_ap=fidx_sb[:, c0 * 8 : (c0 + nlo) * 8],
                            num_idxs=nlo * 128,
                            num_idxs_reg=nlo * 128,
                            elem_size=D,
                            single_packet=False,
                        )
                    if nhi:
                        c0 = g["col0"][2]
                        nc.gpsimd.dma_gather(
                            out_ap=T[:, nlo : nlo + nhi, :],
                            in_ap=xp_full[HALF:N, :],
                            idxs_ap=fidx_sb[:, c0 * 8 : (c0 + nhi) * 8],
                            num_idxs=nhi * 128,
                            num_idxs_reg=nhi * 128,
                            elem_size=D,
                            single_packet=False,
                        )

                    # oh/ohT entries: [own entries | lo+hi entries]
                    ne_own, ne_lohi = g["nent_own"], g["nent_lohi"]
                    nent = ne_own + ne_lohi
                    ohg = opool.tile([128, ENT_MAX * 128], BF16, tag="ohg")
                    ohTg = otpool.tile([128, ENT_MAX * 128], BF16, tag="ohTg")
                    if ne_own:
                        e0 = g["e0own"]
                        nc.sync.dma_start(
                            ohg[:, 0 : ne_own * 128],
                            oh_t[:, e0 * 128 : (e0 + ne_own) * 128],
                        )
                        nc.sync.dma_start(
                            ohTg[:, 0 : ne_own * 128],
                            ohT_t[:, e0 * 128 : (e0 + ne_own) * 128],
                        )
                    e0 = g["e0lohi"]
                    nc.sync.dma_start(
                        ohg[:, ne_own * 128 : nent * 128],
                        oh_t[:, e0 * 128 : (e0 + ne_lohi) * 128],
                    )
                    nc.sync.dma_start(
                        ohTg[:, ne_own * 128 : nent * 128],
                        ohT_t[:, e0 * 128 : (e0 + ne_lohi) * 128],
                    )

                    def tcol(j):
                        """(tile, local col) for group column j (own first)."""
                        if j < nown:
                            return ownT[gi], j
                        return T, j - nown

                    # gate: a_dst columns data-dependent on the gathers so the
                    # a_dst matmuls can't hoist into phase A
                    gn = len(blocks)
                    g0 = blocks[0]
                    tz = epool.tile([128, 1], F32, tag="tz")
                    src_gate = T if cbl else ownT[gi]
                    nc.vector.tensor_scalar(tz[:], src_gate[:, 0, 0:1], 0.0, None, OP.mult)
                    adst_g = epool.tile([128, GBLK], BF16, tag="adst_g")
                    nc.vector.tensor_scalar(
                        adst_g[:, 0:gn], adst_sb[:, g0 : g0 + gn], tz[:, 0:1],
                        None, OP.add,
                    )

                    # a_dst lookups: per (column, block) entry, accumulated
                    # into the column's ps_adst slot
                    ps_adst = psapool.tile([128, max(cbg, 1)], F32, space="PSUM")
                    ents = g["entries"]  # {s: [(b, j_seg, eidx_global)]}
                    # group-local column index: own seg cols [0, nown);
                    # lo cols [nown, nown+nlo); hi cols [nown+nlo, cbg)
                    colbase = {0: 0, 1: nown, 2: nown + nlo}

                    def eloc(s, el):
                        """global oh entry index -> column in ohg/ohTg tile"""
                        if s == 0:
                            return el - g["e0own"]
                        return el - g["e0lohi"] + ne_own

                    # first/last entry per column for start/stop flags
                    col_ents = {}
                    for s in (0, 1, 2):
                        for (b, j, el) in ents[s]:
                            col_ents.setdefault(colbase[s] + j, []).append(
                                (b, eloc(s, el))
                            )
                    for col in sorted(col_ents):
                        for idx, (b, el) in enumerate(col_ents[col]):
                            nc.tensor.matmul(
                                ps_adst[:, col : col + 1],
                                lhsT=ohTg[:, el * 128 : (el + 1) * 128],
                                rhs=adst_g[:, b - g0 : b - g0 + 1],
                                start=(idx == 0),
                                stop=(idx == len(col_ents[col]) - 1),
                            )

                    # ee = exp(leakyrelu(a_src + a_dst)) for the whole group
                    adst_bg = epool.tile([128, max(cbg, 1)], BF16, tag="adst_bg")
                    nc.vector.tensor_copy(adst_bg[:, 0:cbg], ps_adst[:, 0:cbg])
                    e1 = epool.tile([128, max(cbg, 1)], BF16, tag="e1")
                    if nown:
                        nc.vector.tensor_tensor(
                            e1[:, 0:nown], ownT[gi][:, 0:nown, jmax],
                            adst_bg[:, 0:nown], OP.add,
                        )
                    if cbl:
                        nc.vector.tensor_tensor(
                            e1[:, nown:cbg], T[:, 0:cbl, jmax],
                            adst_bg[:, nown:cbg], OP.add,
                        )
                    e2 = epool.tile([128, max(cbg, 1)], BF16, tag="e2")
                    nc.vector.tensor_scalar(
                        e2[:, 0:cbg], e1[:, 0:cbg], NEG_SLOPE, None, OP.mult
                    )
                    e3 = epool.tile([128, max(cbg, 1)], BF16, tag="e3")
                    nc.vector.tensor_tensor(
                        e3[:, 0:cbg], e2[:, 0:cbg], e1[:, 0:cbg], OP.max
                    )
                    ee = epool.tile([128, max(cbg, 1)], F32, tag="ee")
                    nc.scalar.activation(ee[:, 0:cbg], e3[:, 0:cbg], AF.Exp)

                    # rhs rows scaled by ee (Scalar engine), ee denom column
                    T2 = fpool.tile([128, CBG_MAX + OWN_MAX, D + 1], BF16, tag="T2")
                    for col in range(cbg):
                        tt, lj = tcol(col)
                        nc.scalar.activation(
                            T2[:, col, 0:D], tt[:, lj, :], AF.Copy,
                            scale=ee[:, col : col + 1],
                        )
                    nc.vector.tensor_copy(T2[:, 0:cbg, D], ee[:, 0:cbg])

                    # per-block scatter: all entries of block b accumulate
                    blk_ents = {b: [] for b in blocks}
                    for s in (0, 1, 2):
                        for (b, j, el) in ents[s]:
                            blk_ents[b].append((colbase[s] + j, eloc(s, el)))
                    for b in blocks:
                        elist = blk_ents[b]
                        rows = 128 if b < NBLK - 1 else LAST_ROWS
                        ps = psopool.tile([128, D + 1], F32, space="PSUM")
                        for k, (col, el) in enumerate(elist):
                            nc.tensor.matmul(
                                ps[:, :],
                                lhsT=ohg[:, el * 128 : (el + 1) * 128],
                                rhs=T2[:, col, 0 : D + 1],
                                start=(k == 0),
                                stop=(k == len(elist) - 1),
                            )
                        recip = epool.tile([128, 1], F32, tag="recip")
                        nc.vector.reciprocal(recip[:], ps[:, D : D + 1])
                        scaled = outpool.tile([128, D], F32, tag="scaled")
                        nc.scalar.activation(
                            scaled[:], ps[:, 0:D], AF.Copy, scale=recip[:, 0:1]
                        )
                        q = outpool.tile([128, D], F32, tag="q")
                        nc.vector.tensor_tensor(q[:], scaled[:], attb_sb[:], OP.mult)
                        qs = epool.tile([128, 1], F32, tag="qs")
                        nc.vector.tensor_reduce(qs[:], q[:], AX.X, OP.add)
                        numer = epool.tile([128, 1], F32, tag="numer")
                        nc.vector.tensor_tensor(
                            numer[:], scaled[:, jmax : jmax + 1], qs[:], OP.subtract
                        )
                        nc.vector.tensor_scalar(
                            scaled[:, jmax : jmax + 1], numer[:], inv_ajmax,
                            None, OP.mult,
                        )
                        resid = outpool.tile([128, D], F32, tag="resid")
                        nc.vector.tensor_tensor(resid[:], scaled[:], xres(b), OP.add)
                        outt = outpool.tile([128, D], F32, tag="outt")
                        nc.scalar.activation(outt[:], resid[:], AF.Relu)
                        nc.sync.dma_start(
                            out_shard[b * 128 : b * 128 + rows, :], outt[:rows, :]
                        )

    nc.compile()
    return nc


def _wrap_idx(idx):
    L = len(idx)
    assert L % 16 == 0
    w = idx.reshape(L // 16, 16).T.astype(np.int16)
    return np.tile(w, (8, 1))


def _host_prep(x, edge_index, ln_gamma, ln_beta, W, att_src, att_dst, bias):
    """Fold parameters; pack edges at (group, segment) granularity."""
    Wt = W.T.astype(np.float64)
    G = ln_gamma.astype(np.float64)[:, None] * Wt
    crow = ln_beta.astype(np.float64) @ Wt
    a_src = att_src.astype(np.float64)
    v_src = G @ a_src
    v_dst = G @ att_dst.astype(np.float64)
    c_dst = float(crow @ att_dst.astype(np.float64))
    biasf = bias.astype(np.float64)

    jmax = int(np.argmax(np.abs(a_src)))
    inv_ajmax = float(1.0 / a_src[jmax])

    wext = np.zeros((D, 129), np.float32)
    wext[:, 0:D] = G.astype(np.float32)
    wext[:, jmax] = v_src.astype(np.float32)
    wext[:, 128] = v_dst.astype(np.float32)
    c2 = np.zeros((129,), np.float32)
    c2[0:D] = (crow + biasf).astype(np.float32)
    c2[jmax] = float((crow + biasf) @ a_src)
    c2[128] = c_dst - float(biasf @ a_src)
    c2b = np.broadcast_to(c2, (128, 129)).copy()

    ident = np.eye(128, dtype=np.float32)
    attb = np.broadcast_to(a_src.astype(np.float32), (128, 128)).copy()
    attb[:, jmax] = 0.0

    src = np.concatenate([edge_index[0], np.arange(N, dtype=np.int64)]).astype(np.int64)
    dst = np.concatenate([edge_index[1], np.arange(N, dtype=np.int64)]).astype(np.int64)
    core = dst // SHARD
    local = dst - core * SHARD
    blk = local // 128
    own = (src // SHARD) == core
    seg = np.where(own, 0, 1 + (src >= HALF))
    gid = blk // GBLK
    ngrp = (NBLK + GBLK - 1) // GBLK
    # sort by (core, group, seg, block)
    key = ((core * ngrp + gid) * 3 + seg) * NBLK + blk
    order = np.argsort(key, kind="stable")
    src, dst, seg, core, blk, gid = (
        src[order], dst[order], seg[order], core[order], blk[order], gid[order]
    )
    # per (core, group, seg) counts
    kgs = (core * ngrp + gid) * 3 + seg
    cnt_gs = np.bincount(kgs, minlength=NCORES * ngrp * 3).reshape(NCORES, ngrp, 3)
    ncols_gs = -(-cnt_gs.max(axis=0) // 128)               # [ngrp, 3]

    # per (core, group, seg, block) counts -> per-core block spans in cols
    kgb = ((core * ngrp + gid) * 3 + seg) * NBLK + blk
    cnt_gb = np.bincount(kgb, minlength=NCORES * ngrp * 3 * NBLK).reshape(
        NCORES, ngrp, 3, NBLK
    )

    # build layout (static, cross-core): per group/seg, per block the column
    # span [min-over-cores floor(start/128), max-over-cores ceil(end/128))
    groups = []
    col_cursor = {}
    # global fidx column numbering: all own segs (by group) first, then per
    # group lo and hi
    col0_own = {}
    cur = 0
    for gi in range(ngrp):
        col0_own[gi] = cur
        cur += int(ncols_gs[gi, 0])
    col0_lohi = {}
    for gi in range(ngrp):
        col0_lohi[(gi, 1)] = cur
        cur += int(ncols_gs[gi, 1])
        col0_lohi[(gi, 2)] = cur
        cur += int(ncols_gs[gi, 2])
    ncols_tot = cur

    # entry numbering: all own entries (by group) first, then per group lo+hi
    entries_all = {}
    ent_cursor = 0
    e0own = {}
    e0lohi = {}
    for phase in (0, 1):
        for gi in range(ngrp):
            blocks = list(range(gi * GBLK, min(NBLK, gi * GBLK + GBLK)))
            segs = (0,) if phase == 0 else (1, 2)
            if phase == 0:
                e0own[gi] = ent_cursor
            else:
                e0lohi[gi] = ent_cursor
            for s in segs:
                ncol = int(ncols_gs[gi, s])
                if ncol == 0:
                    entries_all[(gi, s)] = []
                    continue
                # per-core start offsets of each block within the seg
                starts_c = np.zeros((NCORES, len(blocks) + 1), np.int64)
                for ci in range(NCORES):
                    starts_c[ci, 1:] = np.cumsum(cnt_gb[ci, gi, s, blocks])
                ents = []
                for bi, b in enumerate(blocks):
                    lo_col = int(starts_c[:, bi].min() // 128)
                    hi_col = int(-(-starts_c[:, bi + 1].max() // 128))
                    hi_col = min(hi_col, ncol)
                    if starts_c[:, bi + 1].max() == starts_c[:, bi].min():
                        continue
                    for j in range(lo_col, hi_col):
                        ents.append((b, j, ent_cursor))
                        ent_cursor += 1
                entries_all[(gi, s)] = ents
    nent_tot = ent_cursor

    layout_groups = []
    for gi in range(ngrp):
        blocks = list(range(gi * GBLK, min(NBLK, gi * GBLK + GBLK)))
        layout_groups.append(
            {
                "blocks": blocks,
                "ncols": {s: int(ncols_gs[gi, s]) for s in range(3)},
                "col0": {0: col0_own[gi], 1: col0_lohi[(gi, 1)], 2: col0_lohi[(gi, 2)]},
                "entries": {s: entries_all[(gi, s)] for s in range(3)},
                "e0own": e0own[gi],
                "e0lohi": e0lohi[gi],
                "nent_own": len(entries_all[(gi, 0)]),
                "nent_lohi": len(entries_all[(gi, 1)]) + len(entries_all[(gi, 2)]),
            }
        )
    layout = (layout_groups, {"ncols_tot": ncols_tot, "nent_tot": nent_tot})

    # ---- per-core tables -------------------------------------------------
    feat_idx = np.zeros((NCORES, ncols_tot * 128), np.int16)
    oh = np.zeros((NCORES, 128, nent_tot * 128), NPBF)
    ohT = np.zeros((NCORES, 128, nent_tot * 128), NPBF)

    # per-core edge ranges for (core, group, seg): prefix over sorted arrays
    k_sorted = kgs  # sorted already by construction
    starts_gs = np.zeros(NCORES * ngrp * 3 + 1, np.int64)
    starts_gs[1:] = np.cumsum(cnt_gs.reshape(-1))

    for c in range(NCORES):
        for gi in range(ngrp):
            blocks = list(range(gi * GBLK, min(NBLK, gi * GBLK + GBLK)))
            for s in range(3):
                i0 = starts_gs[(c * ngrp + gi) * 3 + s]
                i1 = starts_gs[(c * ngrp + gi) * 3 + s + 1]
                n = int(i1 - i0)
                if n == 0:
                    continue
                colbase = col0_own[gi] if s == 0 else col0_lohi[(gi, s)]
                k = np.arange(n)
                base = c * SHARD if s == 0 else (0 if s == 1 else HALF)
                feat_idx[c, colbase * 128 + k] = (src[i0:i1] - base).astype(np.int16)
                # emit oh entries: edge at position k -> (col k//128, part
                # k%128), block blk[i0+k], local row dl
                dl = (dst[i0:i1] - (blk[i0:i1] * 128 + core[i0:i1] * SHARD)).astype(
                    np.int64
                )
                p = k % 128
                col = k // 128
                bb = blk[i0:i1]
                ent_of = {}
                for (b, j, el) in entries_all[(gi, s)]:
                    ent_of[(b, j)] = el
                el_arr = np.array(
                    [ent_of[(int(bb[t]), int(col[t]))] for t in range(n)],
                    dtype=np.int64,
                )
                oh[c, p, el_arr * 128 + dl] = 1.0
                ohT[c, dl, el_arr * 128 + p] = 1.0

    in_maps = []
    for c in range(NCORES):
        xs = np.zeros((PAD_SHARD, D), np.float32)
        xs[0:SHARD] = x[c * SHARD : (c + 1) * SHARD]
        in_maps.append(
            {
                "x_shard": xs,
                "wext": wext.astype(NPBF),
                "c2b": c2b,
                "ident": ident,
                "attb": attb,
                "feat_idx": _wrap_idx(feat_idx[c]),
                "oh_t": np.ascontiguousarray(oh[c]),
                "ohT_t": np.ascontiguousarray(ohT[c]),
            }
        )
    return layout, jmax, inv_ajmax, in_maps


def _layout_key(layout):
    groups, tot = layout
    parts = [tot["ncols_tot"], tot["nent_tot"]]
    for g in groups:
        parts.append(
            (
                tuple(g["blocks"]),
                tuple(sorted(g["ncols"].items())),
                tuple(sorted(g["col0"].items())),
                tuple((s, tuple(g["entries"][s])) for s in range(3)),
                g["e0own"],
                g["e0lohi"],
            )
        )
    return tuple(parts)


_PROGRAM_CACHE = {}


def kernel(x, edge_index, edge_attr, h, batch, ln_gamma, ln_beta, W, att_src,
           att_dst, bias):
    x = np.asarray(x, dtype=np.float32)
    edge_index = np.asarray(edge_index)
    h = np.asarray(h)
    ln_gamma = np.asarray(ln_gamma, dtype=np.float32)
    ln_beta = np.asarray(ln_beta, dtype=np.float32)
    W = np.asarray(W, dtype=np.float32)
    att_src = np.asarray(att_src, dtype=np.float32)
    att_dst = np.asarray(att_dst, dtype=np.float32)
    bias = np.asarray(bias, dtype=np.float32)

    layout, jmax, inv_ajmax, in_maps = _host_prep(
        x, edge_index, ln_gamma, ln_beta, W, att_src, att_dst, bias
    )
    key = (_layout_key(layout), jmax)
    if key not in _PROGRAM_CACHE:
        _PROGRAM_CACHE[key] = _build_program(layout, jmax, inv_ajmax)
    nc = _PROGRAM_CACHE[key]

    res = run_bass_kernel_spmd(nc, in_maps, core_ids=list(range(NCORES)))
    out = np.concatenate([res.results[c]["out_shard"] for c in range(NCORES)], axis=0)
    return out, h


# revision 11
# speedup vs baseline: 2.4183x; 1.0790x over previous
"""GAT layer (LayerNorm -> GATConv(heads=1) -> residual ReLU) on 8 trn2 NeuronCores.

Sharding: destination-node parallel. Each core owns N/8 nodes: it computes the
node transform for its shard, shards are AllGathered (bf16, 256 B rows), and
each core processes the edges whose destination falls in its shard.

Per-edge source records are fetched with 256 B non-transposed dma_gathers.
SWDGE descriptor generation (~7.8 ns/row, Q7-bound) is the critical resource:
  * slots are packed at (group, segment) granularity -- columns may straddle
    two dst blocks; boundary columns simply get one extra accumulating
    matmul per extra block (one-hot tables are emitted per (column, block)).
  * own-shard edges (src in the core's own shard) gather from the local
    table and are issued before the AllGather, hiding their descgen.
  * the Vector engine avoids 2-port DVE modes during phase B (they lock
    GpSimd out of SBUF): one-hot operands are host-precomputed and streamed,
    per-edge ee scaling runs on the Scalar engine.

The gathered row packs a_src INTO the feature vector: row[jmax] =
sum_f att_src[f]*xp[f] with jmax = argmax|att_src|; the aggregated feature
jmax is recovered post-scatter from the same linear identity. a_dst is looked
up on the tensor engine (ohT[r,e] one-hot times the block's a_dst column).
ee = exp(leakyrelu(a_src + a_dst)); the scatter-add is a one-hot matmul with
an ee column in the rhs producing the softmax denominator.
"""

import ml_dtypes
import numpy as np

import concourse.bacc as bacc
import concourse.mybir as mybir
import concourse.tile as tile
from concourse.bass_utils import run_bass_kernel_spmd

F32 = mybir.dt.float32
BF16 = mybir.dt.bfloat16
I16 = mybir.dt.int16
AX = mybir.AxisListType
OP = mybir.AluOpType
AF = mybir.ActivationFunctionType
NPBF = ml_dtypes.bfloat16
NPF8 = ml_dtypes.float8_e4m3
FP8 = mybir.dt.float8e4

N = 50000
D = 128
E = 600000
NCORES = 8
SHARD = N // NCORES            # 6250
NBLK = (SHARD + 127) // 128    # 49 dst blocks per core
PAD_SHARD = NBLK * 128         # 6272
LAST_ROWS = SHARD - (NBLK - 1) * 128  # 106
HALF = 32768
NEG_SLOPE = 0.2
LN_EPS = 1e-5
GBLK = 4                       # dst blocks per gather group
ABLK = 4                       # blocks per phase-A giga-iteration


def _build_program(layout, jmax, inv_ajmax):
    """layout: static per-group description (shared by all cores):
    list of dicts with keys
      blocks:   [b...]
      ncols:    {s: cols for seg s}            (s in 0=own,1=lo,2=hi)
      col0:     {s: global fidx column of seg s's first column}
      entries:  {s: [(b, j, eidx), ...]}       j = col within seg,
                                               eidx = global oh-entry index
      e0own / e0lohi: global entry range starts for the group's own / lo+hi
                      entry blocks (own entries contiguous; lo+hi contiguous)
      nent_own / nent_lohi
    plus layout_tot = dict(ncols_tot, nent_tot).
    """
    groups, tot = layout
    CBG_MAX = max(g["ncols"][1] + g["ncols"][2] for g in groups)
    OWN_MAX = max(g["ncols"][0] for g in groups)
    ENT_MAX = max(g["nent_own"] + g["nent_lohi"] for g in groups)
    CB = tot["ncols_tot"]
    NENT = tot["nent_tot"]

    nc = bacc.Bacc("TRN2", num_devices=NCORES, debug=False)

    x_shard = nc.dram_tensor("x_shard", [PAD_SHARD, D], F32, kind="ExternalInput")
    wext = nc.dram_tensor("wext", [D, 129], BF16, kind="ExternalInput")
    c2b = nc.dram_tensor("c2b", [128, 129], F32, kind="ExternalInput")
    ident = nc.dram_tensor("ident", [128, 128], F32, kind="ExternalInput")
    attb = nc.dram_tensor("attb", [128, 128], F32, kind="ExternalInput")
    feat_idx = nc.dram_tensor("feat_idx", [128, CB * 8], I16, kind="ExternalInput")
    oh_t = nc.dram_tensor("oh_t", [128, NENT * 128], FP8, kind="ExternalInput")
    ohT_t = nc.dram_tensor("ohT_t", [128, NENT * 128], FP8, kind="ExternalInput")
    out_shard = nc.dram_tensor("out_shard", [SHARD, D], F32, kind="ExternalOutput")

    with tile.TileContext(nc) as tc:
        with (
            tc.tile_pool(name="dram", bufs=1, space="DRAM") as dram,
            tc.tile_pool(name="consts", bufs=1) as cpool,
            tc.tile_pool(name="xres", bufs=1) as xpool,
            tc.tile_pool(name="ownt", bufs=1) as ownpool,
        ):
            xp_shard = dram.tile([SHARD, D], BF16)
            xp_full = dram.tile([N, D], BF16, addr_space="Shared")

            ident_sb = cpool.tile([128, 128], F32)
            nc.sync.dma_start(ident_sb[:], ident[:, :])
            attb_sb = cpool.tile([128, 128], F32)
            nc.sync.dma_start(attb_sb[:], attb[:, :])
            wext_sb = cpool.tile([D, 129], BF16)
            nc.sync.dma_start(wext_sb[:], wext[:, :])
            c2b_sb = cpool.tile([128, 129], F32)
            nc.sync.dma_start(c2b_sb[:], c2b[:, :])
            eps_sb = cpool.tile([128, 1], F32)
            nc.vector.memset(eps_sb[:], LN_EPS)
            fidx_sb = cpool.tile([128, CB * 8], I16)
            nc.sync.dma_start(fidx_sb[:], feat_idx[:, :])
            adst_sb = cpool.tile([128, NBLK], BF16)

            # x loaded 4 blocks per tile: partition p = row p of each block
            nga = (NBLK + ABLK - 1) // ABLK
            x_tiles = []
            for a in range(nga):
                nb = min(ABLK, NBLK - a * ABLK)
                xt = xpool.tile([128, ABLK, D], F32, tag=f"x4_{a}")
                nc.sync.dma_start(
                    xt[:, 0:nb, :],
                    x_shard[a * ABLK * 128 : (a * ABLK + nb) * 128, :].rearrange(
                        "(a p) b -> p a b", p=128
                    ),
                )
                x_tiles.append(xt)

            def xres(b):
                return x_tiles[b // ABLK][:, b % ABLK, :]

            # ---------------- Phase A: node transform on own shard ---------
            with (
                tc.tile_pool(name="a_small", bufs=6) as spool,
                tc.tile_pool(name="a_sq", bufs=3) as sqpool,
                tc.tile_pool(name="a_xnp", bufs=6) as xnppool,
                tc.tile_pool(name="a_xnpT", bufs=6) as xnptpool,
                tc.tile_pool(name="a_xpe", bufs=6) as xpepool,
                tc.tile_pool(name="a_ps_t", bufs=3, space="PSUM") as psa,
                tc.tile_pool(name="a_ps_m", bufs=3, space="PSUM") as psb,
            ):
                for a in range(nga):
                    nb = min(ABLK, NBLK - a * ABLK)
                    xt = x_tiles[a]
                    sum4 = spool.tile([128, ABLK], F32, tag="sum4")
                    nc.vector.tensor_reduce(sum4[:, 0:nb], xt[:, 0:nb, :], AX.X, OP.add)
                    sq4 = sqpool.tile([128, ABLK, D], F32)
                    nc.scalar.activation(sq4[:, 0:nb, :], xt[:, 0:nb, :], AF.Square)
                    ssq4 = spool.tile([128, ABLK], F32, tag="ssq4")
                    nc.vector.tensor_reduce(ssq4[:, 0:nb], sq4[:, 0:nb, :], AX.X, OP.add)
                    mu4 = spool.tile([128, ABLK], F32, tag="mu4")
                    nc.vector.tensor_scalar(mu4[:, 0:nb], sum4[:, 0:nb], 1.0 / D, None, OP.mult)
                    m24 = spool.tile([128, ABLK], F32, tag="m24")
                    nc.vector.tensor_tensor(m24[:, 0:nb], mu4[:, 0:nb], mu4[:, 0:nb], OP.mult)
                    var4 = spool.tile([128, ABLK], F32, tag="var4")
                    nc.vector.tensor_scalar(
                        var4[:, 0:nb], ssq4[:, 0:nb], 1.0 / D, None, OP.mult
                    )
                    nc.vector.tensor_tensor(
                        var4[:, 0:nb], var4[:, 0:nb], m24[:, 0:nb], OP.subtract
                    )
                    std4 = spool.tile([128, ABLK], F32, tag="std4")
                    nc.scalar.activation(std4[:, 0:nb], var4[:, 0:nb], AF.Sqrt, bias=eps_sb[:, 0:1])
                    rstd4 = spool.tile([128, ABLK], F32, tag="rstd4")
                    nc.vector.reciprocal(rstd4[:, 0:nb], std4[:, 0:nb])
                    for k in range(nb):
                        i = a * ABLK + k
                        rows = 128 if i < NBLK - 1 else LAST_ROWS
                        xnp = xnppool.tile([128, D], F32)
                        nc.vector.tensor_scalar(
                            xnp[:], xt[:, k, :], mu4[:, k : k + 1],
                            rstd4[:, k : k + 1], OP.subtract, OP.mult,
                        )
                        pt = psa.tile([128, 128], F32, space="PSUM")
                        nc.tensor.transpose(pt[:], xnp[:], ident_sb[:])
                        xnpT = xnptpool.tile([128, 128], BF16)
                        nc.scalar.copy(xnpT[:], pt[:])
                        pm = psb.tile([128, 129], F32, space="PSUM")
                        nc.tensor.matmul(
                            pm[:], lhsT=xnpT[:], rhs=wext_sb[:], start=True, stop=True
                        )
                        xpe = xpepool.tile([128, 129], BF16)
                        nc.vector.tensor_tensor(xpe[:], pm[:], c2b_sb[:], OP.add)
                        nc.sync.dma_start(
                            xp_shard[i * 128 : i * 128 + rows, :], xpe[:rows, 0:128]
                        )
                        nc.vector.tensor_copy(adst_sb[:, i : i + 1], xpe[:, 128:129])

            # AllGather triggers first (runs on the CC cores), then the
            # own-shard gathers' descgen overlaps the collective transfer
            nc.gpsimd.collective_compute(
                "AllGather",
                OP.bypass,
                replica_groups=[list(range(NCORES))],
                ins=[xp_shard[:, :]],
                outs=[xp_full[:, :]],
            )
            ownT = []
            for gi, g in enumerate(groups):
                ncol = g["ncols"][0]
                t = ownpool.tile([128, max(ncol, 1), D], BF16, tag=f"ownT{gi}")
                if ncol:
                    c0 = g["col0"][0]
                    nc.gpsimd.dma_gather(
                        out_ap=t[:, 0:ncol, :],
                        in_ap=xp_shard[:, :],
                        idxs_ap=fidx_sb[:, c0 * 8 : (c0 + ncol) * 8],
                        num_idxs=ncol * 128,
                        num_idxs_reg=ncol * 128,
                        elem_size=D,
                        single_packet=False,
                    )
                ownT.append(t)

            # ---------------- Phase B: edge aggregation --------------------
            with (
                tc.tile_pool(name="b_g", bufs=3) as gpool,
                tc.tile_pool(name="b_oh", bufs=2) as opool,
                tc.tile_pool(name="b_ohT", bufs=2) as otpool,
                tc.tile_pool(name="b_f", bufs=2) as fpool,
                tc.tile_pool(name="b_e", bufs=3) as epool,
                tc.tile_pool(name="b_o", bufs=4) as outpool,
                tc.tile_pool(name="b_pso", bufs=4, space="PSUM") as psopool,
                tc.tile_pool(name="b_psa", bufs=2, space="PSUM") as psapool,
            ):
                for gi, g in enumerate(groups):
                    blocks = g["blocks"]
                    nown = g["ncols"][0]
                    nlo, nhi = g["ncols"][1], g["ncols"][2]
                    cbl = nlo + nhi          # lo+hi cols in T
                    cbg = nown + cbl         # total cols this group
                    T = gpool.tile([128, CBG_MAX, D], BF16, tag="T")
                    if nlo:
                        c0 = g["col0"][1]
                        nc.gpsimd.dma_gather(
                            out_ap=T[:, 0:nlo, :],
                            in_ap=xp_full[0:HALF, :],
                            idxs_ap=fidx_sb[:, c0 * 8 : (c0 + nlo) * 8],
                            num_idxs=nlo * 128,
                            num_idxs_reg=nlo * 128,
                            elem_size=D,
                            single_packet=False,
                        )
                    if nhi:
                        c0 = g["col0"][2]
                        nc.gpsimd.dma_gather(
                            out_ap=T[:, nlo : nlo + nhi, :],
                            in_ap=xp_full[HALF:N, :],
                            idxs_ap=fidx_sb[:, c0 * 8 : (c0 + nhi) * 8],
                            num_idxs=nhi * 128,
                            num_idxs_reg=nhi * 128,
                            elem_size=D,
                            single_packet=False,
                        )

                    # oh/ohT entries: [own entries | lo+hi entries]
                    ne_own, ne_lohi = g["nent_own"], g["nent_lohi"]
                    nent = ne_own + ne_lohi
                    ohg = opool.tile([128, ENT_MAX * 128], FP8, tag="ohg")
                    ohTg = otpool.tile([128, ENT_MAX * 128], FP8, tag="ohTg")
                    if ne_own:
                        e0 = g["e0own"]
                        nc.sync.dma_start(
                            ohg[:, 0 : ne_own * 128],
                            oh_t[:, e0 * 128 : (e0 + ne_own) * 128],
                        )
                        nc.sync.dma_start(
                            ohTg[:, 0 : ne_own * 128],
                            ohT_t[:, e0 * 128 : (e0 + ne_own) * 128],
                        )
                    e0 = g["e0lohi"]
                    nc.sync.dma_start(
                        ohg[:, ne_own * 128 : nent * 128],
                        oh_t[:, e0 * 128 : (e0 + ne_lohi) * 128],
                    )
                    nc.sync.dma_start(
                        ohTg[:, ne_own * 128 : nent * 128],
                        ohT_t[:, e0 * 128 : (e0 + ne_lohi) * 128],
                    )

                    def tcol(j):
                        """(tile, local col) for group column j (own first)."""
                        if j < nown:
                            return ownT[gi], j
                        return T, j - nown

                    # gate: a_dst columns data-dependent on the gathers so the
                    # a_dst matmuls can't hoist into phase A
                    gn = len(blocks)
                    g0 = blocks[0]
                    tz = epool.tile([128, 1], F32, tag="tz")
                    src_gate = T if cbl else ownT[gi]
                    nc.vector.tensor_scalar(tz[:], src_gate[:, 0, 0:1], 0.0, None, OP.mult)
                    adst_g = epool.tile([128, GBLK], BF16, tag="adst_g")
                    nc.vector.tensor_scalar(
                        adst_g[:, 0:gn], adst_sb[:, g0 : g0 + gn], tz[:, 0:1],
                        None, OP.add,
                    )

                    # a_dst lookups: per (column, block) entry, accumulated
                    # into the column's ps_adst slot
                    ps_adst = psapool.tile([128, max(cbg, 1)], F32, space="PSUM")
                    ents = g["entries"]  # {s: [(b, j_seg, eidx_global)]}
                    # group-local column index: own seg cols [0, nown);
                    # lo cols [nown, nown+nlo); hi cols [nown+nlo, cbg)
                    colbase = {0: 0, 1: nown, 2: nown + nlo}

                    def eloc(s, el):
                        """global oh entry index -> column in ohg/ohTg tile"""
                        if s == 0:
                            return el - g["e0own"]
                        return el - g["e0lohi"] + ne_own

                    # first/last entry per column for start/stop flags
                    col_ents = {}
                    for s in (0, 1, 2):
                        for (b, j, el) in ents[s]:
                            col_ents.setdefault(colbase[s] + j, []).append(
                                (b, eloc(s, el))
                            )
                    for col in sorted(col_ents):
                        for idx, (b, el) in enumerate(col_ents[col]):
                            nc.tensor.matmul(
                                ps_adst[:, col : col + 1],
                                lhsT=ohTg[:, el * 128 : (el + 1) * 128],
                                rhs=adst_g[:, b - g0 : b - g0 + 1],
                                start=(idx == 0),
                                stop=(idx == len(col_ents[col]) - 1),
                            )

                    # ee = exp(leakyrelu(a_src + a_dst)) for the whole group
                    adst_bg = epool.tile([128, max(cbg, 1)], BF16, tag="adst_bg")
                    nc.vector.tensor_copy(adst_bg[:, 0:cbg], ps_adst[:, 0:cbg])
                    e1 = epool.tile([128, max(cbg, 1)], BF16, tag="e1")
                    if nown:
                        nc.vector.tensor_tensor(
                            e1[:, 0:nown], ownT[gi][:, 0:nown, jmax],
                            adst_bg[:, 0:nown], OP.add,
                        )
                    if cbl:
                        nc.vector.tensor_tensor(
                            e1[:, nown:cbg], T[:, 0:cbl, jmax],
                            adst_bg[:, nown:cbg], OP.add,
                        )
                    e2 = epool.tile([128, max(cbg, 1)], BF16, tag="e2")
                    nc.vector.tensor_scalar(
                        e2[:, 0:cbg], e1[:, 0:cbg], NEG_SLOPE, None, OP.mult
                    )
                    e3 = epool.tile([128, max(cbg, 1)], BF16, tag="e3")
                    nc.vector.tensor_tensor(
                        e3[:, 0:cbg], e2[:, 0:cbg], e1[:, 0:cbg], OP.max
                    )
                    ee = epool.tile([128, max(cbg, 1)], F32, tag="ee")
                    nc.scalar.activation(ee[:, 0:cbg], e3[:, 0:cbg], AF.Exp)

                    # rhs rows scaled by ee (Scalar engine), ee denom column
                    T2 = fpool.tile([128, CBG_MAX + OWN_MAX, D + 1], BF16, tag="T2")
                    for col in range(cbg):
                        tt, lj = tcol(col)
                        if col % 2 == 0:
                            nc.scalar.activation(
                                T2[:, col, 0:D], tt[:, lj, :], AF.Copy,
                                scale=ee[:, col : col + 1],
                            )
                        else:
                            nc.vector.tensor_scalar(
                                T2[:, col, 0:D], tt[:, lj, :],
                                ee[:, col : col + 1], None, OP.mult,
                            )
                    nc.vector.tensor_copy(T2[:, 0:cbg, D], ee[:, 0:cbg])

                    # per-block scatter: all entries of block b accumulate
                    blk_ents = {b: [] for b in blocks}
                    for s in (0, 1, 2):
                        for (b, j, el) in ents[s]:
                            blk_ents[b].append((colbase[s] + j, eloc(s, el)))
                    for b in blocks:
                        elist = blk_ents[b]
                        rows = 128 if b < NBLK - 1 else LAST_ROWS
                        ps = psopool.tile([128, D + 1], F32, space="PSUM")
                        for k, (col, el) in enumerate(elist):
                            nc.tensor.matmul(
                                ps[:, :],
                                lhsT=ohg[:, el * 128 : (el + 1) * 128],
                                rhs=T2[:, col, 0 : D + 1],
                                start=(k == 0),
                                stop=(k == len(elist) - 1),
                            )
                        recip = epool.tile([128, 1], F32, tag="recip")
                        nc.vector.reciprocal(recip[:], ps[:, D : D + 1])
                        scaled = outpool.tile([128, D], F32, tag="scaled")
                        nc.scalar.activation(
                            scaled[:], ps[:, 0:D], AF.Copy, scale=recip[:, 0:1]
                        )
                        q = outpool.tile([128, D], F32, tag="q")
                        nc.vector.tensor_tensor(q[:], scaled[:], attb_sb[:], OP.mult)
                        qs = epool.tile([128, 1], F32, tag="qs")
                        nc.vector.tensor_reduce(qs[:], q[:], AX.X, OP.add)
                        numer = epool.tile([128, 1], F32, tag="numer")
                        nc.vector.tensor_tensor(
                            numer[:], scaled[:, jmax : jmax + 1], qs[:], OP.subtract
                        )
                        nc.vector.tensor_scalar(
                            scaled[:, jmax : jmax + 1], numer[:], inv_ajmax,
                            None, OP.mult,
                        )
                        resid = outpool.tile([128, D], F32, tag="resid")
                        nc.vector.tensor_tensor(resid[:], scaled[:], xres(b), OP.add)
                        outt = outpool.tile([128, D], F32, tag="outt")
                        nc.scalar.activation(outt[:], resid[:], AF.Relu)
                        nc.sync.dma_start(
                            out_shard[b * 128 : b * 128 + rows, :], outt[:rows, :]
                        )

    nc.compile()
    return nc


def _wrap_idx(idx):
    L = len(idx)
    assert L % 16 == 0
    w = idx.reshape(L // 16, 16).T.astype(np.int16)
    return np.tile(w, (8, 1))


def _host_prep(x, edge_index, ln_gamma, ln_beta, W, att_src, att_dst, bias):
    """Fold parameters; pack edges at (group, segment) granularity."""
    Wt = W.T.astype(np.float64)
    G = ln_gamma.astype(np.float64)[:, None] * Wt
    crow = ln_beta.astype(np.float64) @ Wt
    a_src = att_src.astype(np.float64)
    v_src = G @ a_src
    v_dst = G @ att_dst.astype(np.float64)
    c_dst = float(crow @ att_dst.astype(np.float64))
    biasf = bias.astype(np.float64)

    jmax = int(np.argmax(np.abs(a_src)))
    inv_ajmax = float(1.0 / a_src[jmax])

    wext = np.zeros((D, 129), np.float32)
    wext[:, 0:D] = G.astype(np.float32)
    wext[:, jmax] = v_src.astype(np.float32)
    wext[:, 128] = v_dst.astype(np.float32)
    c2 = np.zeros((129,), np.float32)
    c2[0:D] = (crow + biasf).astype(np.float32)
    c2[jmax] = float((crow + biasf) @ a_src)
    c2[128] = c_dst - float(biasf @ a_src)
    c2b = np.broadcast_to(c2, (128, 129)).copy()

    ident = np.eye(128, dtype=np.float32)
    attb = np.broadcast_to(a_src.astype(np.float32), (128, 128)).copy()
    attb[:, jmax] = 0.0

    src = np.concatenate([edge_index[0], np.arange(N, dtype=np.int64)]).astype(np.int64)
    dst = np.concatenate([edge_index[1], np.arange(N, dtype=np.int64)]).astype(np.int64)
    core = dst // SHARD
    local = dst - core * SHARD
    blk = local // 128
    own = (src // SHARD) == core
    seg = np.where(own, 0, 1 + (src >= HALF))
    gid = blk // GBLK
    ngrp = (NBLK + GBLK - 1) // GBLK
    # sort by (core, group, seg, block)
    key = ((core * ngrp + gid) * 3 + seg) * NBLK + blk
    order = np.argsort(key, kind="stable")
    src, dst, seg, core, blk, gid = (
        src[order], dst[order], seg[order], core[order], blk[order], gid[order]
    )
    # per (core, group, seg) counts
    kgs = (core * ngrp + gid) * 3 + seg
    cnt_gs = np.bincount(kgs, minlength=NCORES * ngrp * 3).reshape(NCORES, ngrp, 3)
    ncols_gs = -(-cnt_gs.max(axis=0) // 128)               # [ngrp, 3]

    # per (core, group, seg, block) counts -> per-core block spans in cols
    kgb = ((core * ngrp + gid) * 3 + seg) * NBLK + blk
    cnt_gb = np.bincount(kgb, minlength=NCORES * ngrp * 3 * NBLK).reshape(
        NCORES, ngrp, 3, NBLK
    )

    # build layout (static, cross-core): per group/seg, per block the column
    # span [min-over-cores floor(start/128), max-over-cores ceil(end/128))
    groups = []
    col_cursor = {}
    # global fidx column numbering: all own segs (by group) first, then per
    # group lo and hi
    col0_own = {}
    cur = 0
    for gi in range(ngrp):
        col0_own[gi] = cur
        cur += int(ncols_gs[gi, 0])
    col0_lohi = {}
    for gi in range(ngrp):
        col0_lohi[(gi, 1)] = cur
        cur += int(ncols_gs[gi, 1])
        col0_lohi[(gi, 2)] = cur
        cur += int(ncols_gs[gi, 2])
    ncols_tot = cur

    # entry numbering: all own entries (by group) first, then per group lo+hi
    entries_all = {}
    ent_cursor = 0
    e0own = {}
    e0lohi = {}
    for phase in (0, 1):
        for gi in range(ngrp):
            blocks = list(range(gi * GBLK, min(NBLK, gi * GBLK + GBLK)))
            segs = (0,) if phase == 0 else (1, 2)
            if phase == 0:
                e0own[gi] = ent_cursor
            else:
                e0lohi[gi] = ent_cursor
            for s in segs:
                ncol = int(ncols_gs[gi, s])
                if ncol == 0:
                    entries_all[(gi, s)] = []
                    continue
                # per-core start offsets of each block within the seg
                starts_c = np.zeros((NCORES, len(blocks) + 1), np.int64)
                for ci in range(NCORES):
                    starts_c[ci, 1:] = np.cumsum(cnt_gb[ci, gi, s, blocks])
                ents = []
                for bi, b in enumerate(blocks):
                    lo_col = int(starts_c[:, bi].min() // 128)
                    hi_col = int(-(-starts_c[:, bi + 1].max() // 128))
                    hi_col = min(hi_col, ncol)
                    if starts_c[:, bi + 1].max() == starts_c[:, bi].min():
                        continue
                    for j in range(lo_col, hi_col):
                        ents.append((b, j, ent_cursor))
                        ent_cursor += 1
                entries_all[(gi, s)] = ents
    nent_tot = ent_cursor

    layout_groups = []
    for gi in range(ngrp):
        blocks = list(range(gi * GBLK, min(NBLK, gi * GBLK + GBLK)))
        layout_groups.append(
            {
                "blocks": blocks,
                "ncols": {s: int(ncols_gs[gi, s]) for s in range(3)},
                "col0": {0: col0_own[gi], 1: col0_lohi[(gi, 1)], 2: col0_lohi[(gi, 2)]},
                "entries": {s: entries_all[(gi, s)] for s in range(3)},
                "e0own": e0own[gi],
                "e0lohi": e0lohi[gi],
                "nent_own": len(entries_all[(gi, 0)]),
                "nent_lohi": len(entries_all[(gi, 1)]) + len(entries_all[(gi, 2)]),
            }
        )
    layout = (layout_groups, {"ncols_tot": ncols_tot, "nent_tot": nent_tot})

    # ---- per-core tables -------------------------------------------------
    feat_idx = np.zeros((NCORES, ncols_tot * 128), np.int16)
    oh = np.zeros((NCORES, 128, nent_tot * 128), NPF8)
    ohT = np.zeros((NCORES, 128, nent_tot * 128), NPF8)

    # per-core edge ranges for (core, group, seg): prefix over sorted arrays
    k_sorted = kgs  # sorted already by construction
    starts_gs = np.zeros(NCORES * ngrp * 3 + 1, np.int64)
    starts_gs[1:] = np.cumsum(cnt_gs.reshape(-1))

    for c in range(NCORES):
        for gi in range(ngrp):
            blocks = list(range(gi * GBLK, min(NBLK, gi * GBLK + GBLK)))
            for s in range(3):
                i0 = starts_gs[(c * ngrp + gi) * 3 + s]
                i1 = starts_gs[(c * ngrp + gi) * 3 + s + 1]
                n = int(i1 - i0)
                if n == 0:
                    continue
                colbase = col0_own[gi] if s == 0 else col0_lohi[(gi, s)]
                k = np.arange(n)
                base = c * SHARD if s == 0 else (0 if s == 1 else HALF)
                feat_idx[c, colbase * 128 + k] = (src[i0:i1] - base).astype(np.int16)
                # emit oh entries: edge at position k -> (col k//128, part
                # k%128), block blk[i0+k], local row dl
                dl = (dst[i0:i1] - (blk[i0:i1] * 128 + core[i0:i1] * SHARD)).astype(
                    np.int64
                )
                p = k % 128
                col = k // 128
                bb = blk[i0:i1]
                ent_of = {}
                for (b, j, el) in entries_all[(gi, s)]:
                    ent_of[(b, j)] = el
                el_arr = np.array(
                    [ent_of[(int(bb[t]), int(col[t]))] for t in range(n)],
                    dtype=np.int64,
                )
                oh[c, p, el_arr * 128 + dl] = 1.0
                ohT[c, dl, el_arr * 128 + p] = 1.0

    in_maps = []
    for c in range(NCORES):
        xs = np.zeros((PAD_SHARD, D), np.float32)
        xs[0:SHARD] = x[c * SHARD : (c + 1) * SHARD]
        in_maps.append(
            {
                "x_shard": xs,
                "wext": wext.astype(NPBF),
                "c2b": c2b,
                "ident": ident,
                "attb": attb,
                "feat_idx": _wrap_idx(feat_idx[c]),
                "oh_t": np.ascontiguousarray(oh[c]),
                "ohT_t": np.ascontiguousarray(ohT[c]),
            }
        )
    return layout, jmax, inv_ajmax, in_maps


def _layout_key(layout):
    groups, tot = layout
    parts = [tot["ncols_tot"], tot["nent_tot"]]
    for g in groups:
        parts.append(
            (
                tuple(g["blocks"]),
                tuple(sorted(g["ncols"].items())),
                tuple(sorted(g["col0"].items())),
                tuple((s, tuple(g["entries"][s])) for s in range(3)),
                g["e0own"],
                g["e0lohi"],
            )
        )
    return tuple(parts)


_PROGRAM_CACHE = {}


def kernel(x, edge_index, edge_attr, h, batch, ln_gamma, ln_beta, W, att_src,
           att_dst, bias):
    x = np.asarray(x, dtype=np.float32)
    edge_index = np.asarray(edge_index)
    h = np.asarray(h)
    ln_gamma = np.asarray(ln_gamma, dtype=np.float32)
    ln_beta = np.asarray(ln_beta, dtype=np.float32)
    W = np.asarray(W, dtype=np.float32)
    att_src = np.asarray(att_src, dtype=np.float32)
    att_dst = np.asarray(att_dst, dtype=np.float32)
    bias = np.asarray(bias, dtype=np.float32)

    layout, jmax, inv_ajmax, in_maps = _host_prep(
        x, edge_index, ln_gamma, ln_beta, W, att_src, att_dst, bias
    )
    key = (_layout_key(layout), jmax)
    if key not in _PROGRAM_CACHE:
        _PROGRAM_CACHE[key] = _build_program(layout, jmax, inv_ajmax)
    nc = _PROGRAM_CACHE[key]

    res = run_bass_kernel_spmd(nc, in_maps, core_ids=list(range(NCORES)))
    out = np.concatenate([res.results[c]["out_shard"] for c in range(NCORES)], axis=0)
    return out, h
